# revision 22
# baseline (speedup 1.0000x reference)
"""Winograd F(4,3)-y conv3d, v4: kk-major, chunked DMA, single X tensor.

Per-core work: 4 z-tiles x 3 planes, N=432 per matmul, 6 winograd
m-terms (kk) x 6 passes covering the 9 (dz,dx) taps as 3 x-duals
(128-contraction using the +1x-shifted upper half) + 3 singles (the
(1,2) single reads the upper half at dx=1 via tile_position (64,0)).

Key structure vs the 56.7us baseline:
- ONE HBM X tensor xd = [T; T(+1x)] (3.35MB, vs 7.7MB) DMA'd in 6
  per-kk chunks on the SP ring; W on the ACT ring. No on-chip copies.
- kk-major matmul order: all 4 z-tiles consume chunk kk right after it
  lands, so the PE starts ~4us in and chunk buffers free early -- the
  For_i hardware loop pipelines across iterations even with static
  tile buffers.
- Inverse transform fused into evacuation: ACT copies m0/m1/m3/m5 out
  of PSUM, DVE forms a/b/p/q with one PSUM operand each (HW limit:
  max one PSUM input per DVE op), scalar_tensor_tensor fuses the
  *2/*4/*8 scaled adds, intermediates bf16 (2x DVE rate).
"""

import sys

if "/opt/trn_rl_repo" not in sys.path:
    sys.path.insert(0, "/opt/trn_rl_repo")

import ml_dtypes
import numpy as np

CIN, COUT, K = 64, 128, 3
DHW = 24
ZS = 12  # z planes per core
NPL = 14  # input planes incl halo
PW = 26
NW = 6  # y window count (stride 4, size 6)
NK = 6  # winograd m-terms per window
N_CORES = 8
ZT = 3  # z planes per tile
ZTILES = (0, 3, 6, 9)
NT = ZT * NW * 24  # 432 cols per matmul

BT = np.array(
    [
        [4, 0, -5, 0, 1, 0],
        [0, -4, -4, 1, 1, 0],
        [0, 4, -4, -1, 1, 0],
        [0, -2, -1, 2, 1, 0],
        [0, 2, -1, -2, 1, 0],
        [0, 4, 0, -5, 0, 1],
    ],
    np.float32,
)
G = np.array(
    [
        [1 / 4, 0, 0],
        [-1 / 6, -1 / 6, -1 / 6],
        [-1 / 6, 1 / 6, -1 / 6],
        [1 / 24, 1 / 12, 1 / 6],
        [1 / 24, -1 / 12, 1 / 6],
        [0, 0, 1],
    ],
    np.float32,
)

# per-(kk,tile) passes: (dz, dx_ap, lo, hi). All passes read xd=[T; T(+1x)]:
# a [0:128] pass computes taps (dz, dx_ap) + (dz, dx_ap+1); a [0:64] pass
# the single lower tap (dz, dx_ap); a [64:128] pass the single upper tap
# (dz, dx_ap+1) via tile_position (64,0).
KPASSES = (
    (0, 0, 0, 128, "dual"),  # (0,0)+(0,1)
    (1, 0, 0, 128, "dual"),  # (1,0)+(1,1)
    (2, 0, 0, 128, "dual"),  # (2,0)+(2,1)
    (0, 2, 0, 64, "lo"),  # single (0,2)
    (1, 1, 0, 128, "up"),  # single (1,2): upper half only, zero lower weights
    (2, 2, 0, 64, "lo"),  # single (2,2)
)
NP_K = len(KPASSES)  # 6


def _elide_redundant_ldweights(nc):
    n_drop = 0
    for f in nc.m.functions:
        for b in f.blocks:
            last_key = None
            drop = []
            for inst in b.instructions:
                if type(inst).__name__ == "InstLdweights":
                    key = (str(inst.ins[0]), str(inst.perf_mode), str(inst.is_transpose))
                    si = inst.sync_info
                    clean = si is None or (len(si.on_wait) == 0 and len(si.on_update) == 0)
                    if key == last_key and clean:
                        drop.append(inst)
                    else:
                        last_key = key
            for inst in drop:
                b.instructions.remove(inst)
            n_drop += len(drop)
    return n_drop


def _build_program(loop_n=None, unroll=False, inner=1):
    import concourse.tile as tile
    from concourse import bacc, mybir

    BF16 = mybir.dt.bfloat16
    F32 = mybir.dt.float32
    MULT = mybir.AluOpType.mult
    ADD = mybir.AluOpType.add

    nc = bacc.Bacc("TRN2")
    xd_in = nc.declare_dram_parameter("xd", [128, NK, NPL, NW, PW], BF16, isOutput=False)
    wk_in = nc.declare_dram_parameter("wk", [128, NK * NP_K, 128], BF16, isOutput=False)
    y_out = nc.declare_dram_parameter("y", [128, ZS, DHW, DHW], BF16, isOutput=True)

    with tile.TileContext(nc) as tc:
        with (
            tc.tile_pool(name="xw", bufs=1) as xw_pool,
            tc.tile_pool(name="ps", bufs=8, space="PSUM") as ps_pool,
            tc.tile_pool(name="ev", bufs=2) as ev_pool,
            tc.tile_pool(name="ob", bufs=4) as ob_pool,
        ):

            def body(_iv=None):
                W = xw_pool.tile([128, NK * NP_K, 128], BF16, name="W", tag="W")
                nc.scalar.dma_start(out=W[:], in_=wk_in[:])
                xdk = []
                for kk in range(NK):
                    xd = xw_pool.tile(
                        [128, NPL, NW, PW], BF16, name=f"xd{kk}", tag=f"xd{kk}"
                    )
                    nc.sync.dma_start(out=xd[:], in_=xd_in[:, kk])
                    xdk.append(xd)

                # psum accumulators, one bank per (tile, kk) group, evacuated
                # progressively so 8 banks suffice in kk-major order
                psq = {}
                evt = {}
                for kk in range(NK):
                    for t, zi in enumerate(ZTILES):
                        ps = ps_pool.tile([128, 512], F32, name="ps", tag="ps")
                        psq[(kk, t)] = ps
                        for p, (dz, dx, lo, hi, _k) in enumerate(KPASSES):
                            j = kk * NP_K + p
                            nc.tensor.matmul(
                                ps[:, :NT],
                                lhsT=W[lo:hi, j, :],
                                rhs=xdk[kk][
                                    lo:hi, zi + dz : zi + dz + ZT, 0:NW, dx : dx + 24
                                ],
                                start=(p == 0),
                                stop=(p == NP_K - 1),
                                skip_group_check=True,
                            )
                    # progressive evacuation: frees the two source banks per op
                    for t in range(4):
                        ps = psq[(kk, t)]

                        def ev(nm, _t=t, dt=BF16):
                            tl = ev_pool.tile(
                                [128, NT], dt, name=f"{nm}{_t}", tag=f"{nm}{_t}"
                            )
                            evt[(nm, _t)] = tl
                            return tl

                        # DVE may read only ONE PSUM operand per op, so odd
                        # m-terms go through an ACT f32 copy first
                        if kk == 0:
                            m0 = ev("m0")
                            nc.scalar.copy(m0[:], ps[:, :NT])
                        elif kk == 1:
                            m1 = ev("m1", dt=F32)
                            nc.scalar.copy(m1[:], ps[:, :NT])
                        elif kk == 2:
                            a = ev("a")
                            nc.vector.tensor_add(
                                a[:], evt[("m1", t)][:], ps[:, :NT]
                            )
                            b = ev("b")
                            nc.vector.tensor_sub(
                                b[:], evt[("m1", t)][:], ps[:, :NT]
                            )
                        elif kk == 3:
                            m3 = ev("m3", dt=F32)
                            nc.scalar.copy(m3[:], ps[:, :NT])
                        elif kk == 4:
                            pp = ev("p")
                            nc.vector.tensor_add(
                                pp[:], evt[("m3", t)][:], ps[:, :NT]
                            )
                            q = ev("q")
                            nc.vector.tensor_sub(
                                q[:], evt[("m3", t)][:], ps[:, :NT]
                            )
                        elif kk == 5:
                            m5 = ev("m5")
                            nc.scalar.copy(m5[:], ps[:, :NT])

                # final combine + store per tile
                for t, zi in enumerate(ZTILES):
                    m0, m5, a, b, pp, q = (
                        evt[(nm, t)] for nm in ("m0", "m5", "a", "b", "p", "q")
                    )
                    ob = ob_pool.tile([128, ZT, NW, 4, 24], BF16, name="ob", tag="ob")
                    u = ev_pool.tile([128, NT], BF16, name=f"u{t}", tag=f"u{t}")
                    nc.vector.tensor_add(u[:], a[:], pp[:])
                    nc.vector.tensor_add(ob[:, :, :, 0, :], u[:], m0[:])
                    nc.vector.scalar_tensor_tensor(
                        ob[:, :, :, 1, :], q[:], 2.0, b[:], MULT, ADD
                    )
                    nc.vector.scalar_tensor_tensor(
                        ob[:, :, :, 2, :], pp[:], 4.0, a[:], MULT, ADD
                    )
                    t2 = ev_pool.tile([128, NT], BF16, name=f"t{t}", tag=f"t{t}")
                    nc.vector.scalar_tensor_tensor(t2[:], q[:], 8.0, b[:], MULT, ADD)
                    nc.vector.tensor_add(ob[:, :, :, 3, :], t2[:], m5[:])
                    nc.sync.dma_start(out=y_out[:, zi : zi + ZT], in_=ob[:])

            if loop_n is not None:
                if unroll:
                    for _k in range(loop_n):
                        body()
                else:
                    with tc.For_i(0, loop_n, 1) as _i:
                        for _j in range(inner):
                            body(_i)
            else:
                body()

    nc.finalize()
    _elide_redundant_ldweights(nc)
    return nc


def _wtap(gw, kk, dz, dx):
    return gw[kk, :, :, dz, dx].T


def _transform_w(weight):
    w = np.asarray(weight, np.float32).reshape(COUT, CIN, K, K, K)
    gw = np.einsum("ky,oczyx->koczx", G, w)  # (6, O, C, 3z, 3x)
    wk = np.zeros((128, NK * NP_K, 128), np.float32)
    for kk in range(NK):
        for p, (dz, dx, lo, hi, kind) in enumerate(KPASSES):
            j = kk * NP_K + p
            if kind == "dual":  # (dz, dx) + (dz, dx+1)
                wk[0:64, j] = _wtap(gw, kk, dz, dx)
                wk[64:128, j] = _wtap(gw, kk, dz, dx + 1)
            elif kind == "lo":  # lower single (dz, dx)
                wk[0:64, j] = _wtap(gw, kk, dz, dx)
            else:  # upper single (dz, dx+1); lower rows stay zero
                wk[64:128, j] = _wtap(gw, kk, dz, dx + 1)
    return wk.astype(ml_dtypes.bfloat16)


def _make_in_maps(x, weight):
    wk = _transform_w(weight)
    x = np.asarray(x, np.float32)
    in_maps = []
    for c in range(N_CORES):
        b, zh = divmod(c, 2)
        z0 = zh * ZS
        xpad = np.zeros((CIN, PW, PW, PW), np.float32)
        xpad[:, 1:25, 1:25, 1:25] = x[b]
        win = xpad[:, z0 : z0 + NPL]  # (64, 14, 26, 26)
        # T[c, k, z, w, x] = sum_j BT[k, j] win[c, z, 4w+j, x]
        wmat = np.lib.stride_tricks.sliding_window_view(win, 6, axis=2)[:, :, ::4][
            :, :, :NW
        ]
        T = np.einsum("kj,czwxj->ckzwx", BT, wmat)  # (64, 6, 14, 6, 26)
        X = np.zeros((128, NK, NPL, NW, PW), np.float32)
        X[0:64] = T
        X[64:128, :, :, :, :-1] = T[:, :, :, :, 1:]  # +1x shift
        in_maps.append({"wk": wk, "xd": X.astype(ml_dtypes.bfloat16)})
    return in_maps


def _gather(results):
    out = np.empty((4, COUT, DHW, DHW, DHW), np.float32)
    for c in range(N_CORES):
        b, zh = divmod(c, 2)
        out[b, :, zh * ZS : (zh + 1) * ZS] = results[c]["y"].astype(np.float32)
    return out


def kernel(x, weight):
    from concourse.bass_utils import run_bass_kernel_spmd

    in_maps = _make_in_maps(x, weight)
    nc = _build_program()
    res = run_bass_kernel_spmd(nc, in_maps, list(range(N_CORES)))
    return _gather(res.results)


def _emulate_core(m):
    """Numpy model of one core incl. bf16 rounding of the AT chain."""
    X = np.asarray(m["xd"], np.float32)
    WK = np.asarray(m["wk"], np.float32)
    bf = lambda a: a.astype(ml_dtypes.bfloat16).astype(np.float32)
    y = np.zeros((128, ZS, DHW, DHW), np.float32)
    for zi in ZTILES:
        ps = np.zeros((NK, 128, NT), np.float32)
        for kk in range(NK):
            for p, (dz, dx, lo, hi, _k) in enumerate(KPASSES):
                j = kk * NP_K + p
                r = X[lo:hi, kk, zi + dz : zi + dz + ZT, 0:NW, dx : dx + 24]
                ps[kk] += WK[lo:hi, j].T @ r.reshape(hi - lo, -1)
        m0 = bf(ps[0])
        m5 = bf(ps[5])
        a = bf(ps[1] + ps[2])
        b_ = bf(ps[1] - ps[2])
        pp = bf(ps[3] + ps[4])
        q = bf(ps[3] - ps[4])
        u = bf(a + pp)
        rows = [bf(u + m0), bf(2 * q + b_), bf(4 * pp + a), bf(bf(8 * q + b_) + m5)]
        yi = np.stack([r.reshape(128, ZT, NW, 24) for r in rows], axis=3)
        y[:, zi : zi + ZT] = yi.reshape(128, ZT, 24, 24)
    return y


if __name__ == "__main__":
    import jax

    sys.path.insert(0, "/root/problem")
    import reference

    cpu = jax.devices("cpu")[0]
    with jax.default_device(cpu):
        inputs = {k: np.asarray(v) for k, v in reference.setup_inputs().items()}
        expected = np.asarray(
            reference.reference(**{k: jax.device_put(v, cpu) for k, v in inputs.items()})
        )
    in_maps = _make_in_maps(inputs["x"], inputs["weight"])
    y = _emulate_core(in_maps[0])
    exp = expected[0][:, 0:ZS]
    err = np.linalg.norm(y - exp) / np.linalg.norm(exp)
    print("emulated core0 rel err:", err)


# revision 26
# speedup vs baseline: 1.0396x; 1.0396x over previous
"""Winograd F(4,3)-y conv3d, v4: kk-major, chunked DMA, single X tensor.

Per-core work: 4 z-tiles x 3 planes, N=432 per matmul, 6 winograd
m-terms (kk) x 6 passes covering the 9 (dz,dx) taps as 3 x-duals
(128-contraction using the +1x-shifted upper half) + 3 singles (the
(1,2) single reads the upper half at dx=1 via tile_position (64,0)).

Key structure vs the 56.7us baseline:
- ONE HBM X tensor xd = [T; T(+1x)] (3.35MB, vs 7.7MB) DMA'd in 6
  per-kk chunks on the SP ring; W on the ACT ring. No on-chip copies.
- kk-major matmul order: all 4 z-tiles consume chunk kk right after it
  lands, so the PE starts ~4us in and chunk buffers free early -- the
  For_i hardware loop pipelines across iterations even with static
  tile buffers.
- Inverse transform fused into evacuation: ACT copies m0/m1/m3/m5 out
  of PSUM, DVE forms a/b/p/q with one PSUM operand each (HW limit:
  max one PSUM input per DVE op), scalar_tensor_tensor fuses the
  *2/*4/*8 scaled adds, intermediates bf16 (2x DVE rate).
"""

import sys

if "/opt/trn_rl_repo" not in sys.path:
    sys.path.insert(0, "/opt/trn_rl_repo")

import ml_dtypes
import numpy as np

CIN, COUT, K = 64, 128, 3
DHW = 24
ZS = 12  # z planes per core
NPL = 14  # input planes incl halo
PW = 26
NW = 6  # y window count (stride 4, size 6)
NK = 6  # winograd m-terms per window
N_CORES = 8
ZT = 3  # z planes per tile
ZTILES = (0, 3, 6, 9)
NT = ZT * NW * 24  # 432 cols per matmul

BT = np.array(
    [
        [4, 0, -5, 0, 1, 0],
        [0, -4, -4, 1, 1, 0],
        [0, 4, -4, -1, 1, 0],
        [0, -2, -1, 2, 1, 0],
        [0, 2, -1, -2, 1, 0],
        [0, 4, 0, -5, 0, 1],
    ],
    np.float32,
)
G = np.array(
    [
        [1 / 4, 0, 0],
        [-1 / 6, -1 / 6, -1 / 6],
        [-1 / 6, 1 / 6, -1 / 6],
        [1 / 24, 1 / 12, 1 / 6],
        [1 / 24, -1 / 12, 1 / 6],
        [0, 0, 1],
    ],
    np.float32,
)

# per-(kk,tile) passes: (dz, dx_ap, lo, hi). All passes read xd=[T; T(+1x)]:
# a [0:128] pass computes taps (dz, dx_ap) + (dz, dx_ap+1); a [0:64] pass
# the single lower tap (dz, dx_ap); a [64:128] pass the single upper tap
# (dz, dx_ap+1) via tile_position (64,0).
KPASSES = (
    (0, 0, 0, 128, "dual"),  # (0,0)+(0,1)
    (1, 0, 0, 128, "dual"),  # (1,0)+(1,1)
    (2, 0, 0, 128, "dual"),  # (2,0)+(2,1)
    (0, 2, 0, 64, "lo"),  # single (0,2)
    (1, 1, 0, 128, "up"),  # single (1,2): upper half only, zero lower weights
    (2, 2, 0, 64, "lo"),  # single (2,2)
)
NP_K = len(KPASSES)  # 6


def _elide_redundant_ldweights(nc):
    n_drop = 0
    for f in nc.m.functions:
        for b in f.blocks:
            last_key = None
            drop = []
            for inst in b.instructions:
                if type(inst).__name__ == "InstLdweights":
                    key = (str(inst.ins[0]), str(inst.perf_mode), str(inst.is_transpose))
                    si = inst.sync_info
                    clean = si is None or (len(si.on_wait) == 0 and len(si.on_update) == 0)
                    if key == last_key and clean:
                        drop.append(inst)
                    else:
                        last_key = key
            for inst in drop:
                b.instructions.remove(inst)
            n_drop += len(drop)
    return n_drop


def _build_program(loop_n=None, unroll=False, inner=1):
    import concourse.tile as tile
    from concourse import bacc, mybir

    BF16 = mybir.dt.bfloat16
    F32 = mybir.dt.float32
    MULT = mybir.AluOpType.mult
    ADD = mybir.AluOpType.add

    nc = bacc.Bacc("TRN2")
    xd_in = nc.declare_dram_parameter("xd", [128, NK, NPL, NW, PW], BF16, isOutput=False)
    wk_in = nc.declare_dram_parameter("wk", [128, NK * NP_K, 128], BF16, isOutput=False)
    y_out = nc.declare_dram_parameter("y", [128, ZS, DHW, DHW], BF16, isOutput=True)

    with tile.TileContext(nc) as tc:
        with (
            tc.tile_pool(name="xw", bufs=1) as xw_pool,
            tc.tile_pool(name="ps", bufs=8, space="PSUM") as ps_pool,
            tc.tile_pool(name="ev", bufs=2) as ev_pool,
            tc.tile_pool(name="ob", bufs=4) as ob_pool,
        ):

            def body(_iv=None):
                W = xw_pool.tile([128, NK * NP_K, 128], BF16, name="W", tag="W")
                nc.scalar.dma_start(out=W[:], in_=wk_in[:])
                # 3 chunks of 2 kk each: few dma_starts (fixed cost ~2us each
                # on the ring) but the first matmuls still start early
                xdk = []
                for ck in range(3):
                    xd = xw_pool.tile(
                        [128, 2, NPL, NW, PW], BF16, name=f"xd{ck}", tag=f"xd{ck}"
                    )
                    nc.sync.dma_start(out=xd[:], in_=xd_in[:, 2 * ck : 2 * ck + 2])
                    xdk.append(xd)

                # psum accumulators, one bank per (tile, kk) group, evacuated
                # progressively so 8 banks suffice in kk-major order
                psq = {}
                evt = {}
                for kk in range(NK):
                    for t, zi in enumerate(ZTILES):
                        ps = ps_pool.tile([128, 512], F32, name="ps", tag="ps")
                        psq[(kk, t)] = ps
                        for p, (dz, dx, lo, hi, _k) in enumerate(KPASSES):
                            j = kk * NP_K + p
                            nc.tensor.matmul(
                                ps[:, :NT],
                                lhsT=W[lo:hi, j, :],
                                rhs=xdk[kk // 2][
                                    lo:hi, kk % 2, zi + dz : zi + dz + ZT, 0:NW,
                                    dx : dx + 24,
                                ],
                                start=(p == 0),
                                stop=(p == NP_K - 1),
                                skip_group_check=True,
                            )
                    # progressive evacuation: frees the two source banks per op
                    for t in range(4):
                        ps = psq[(kk, t)]

                        def ev(nm, _t=t, dt=BF16):
                            tl = ev_pool.tile(
                                [128, NT], dt, name=f"{nm}{_t}", tag=f"{nm}{_t}"
                            )
                            evt[(nm, _t)] = tl
                            return tl

                        # DVE may read only ONE PSUM operand per op, so odd
                        # m-terms go through an ACT f32 copy first
                        if kk == 0:
                            m0 = ev("m0")
                            nc.scalar.copy(m0[:], ps[:, :NT])
                        elif kk == 1:
                            m1 = ev("m1", dt=F32)
                            nc.scalar.copy(m1[:], ps[:, :NT])
                        elif kk == 2:
                            a = ev("a")
                            nc.vector.tensor_add(
                                a[:], evt[("m1", t)][:], ps[:, :NT]
                            )
                            b = ev("b")
                            nc.vector.tensor_sub(
                                b[:], evt[("m1", t)][:], ps[:, :NT]
                            )
                        elif kk == 3:
                            m3 = ev("m3", dt=F32)
                            nc.scalar.copy(m3[:], ps[:, :NT])
                        elif kk == 4:
                            pp = ev("p")
                            nc.vector.tensor_add(
                                pp[:], evt[("m3", t)][:], ps[:, :NT]
                            )
                            q = ev("q")
                            nc.vector.tensor_sub(
                                q[:], evt[("m3", t)][:], ps[:, :NT]
                            )
                        elif kk == 5:
                            m5 = ev("m5")
                            nc.scalar.copy(m5[:], ps[:, :NT])

                # final combine per tile into one merged output buffer,
                # then a single y store (one dma_start instead of four)
                ob = ob_pool.tile([128, ZS, NW, 4, 24], BF16, name="ob", tag="ob")
                for t, zi in enumerate(ZTILES):
                    m0, m5, a, b, pp, q = (
                        evt[(nm, t)] for nm in ("m0", "m5", "a", "b", "p", "q")
                    )
                    u = ev_pool.tile([128, NT], BF16, name=f"u{t}", tag=f"u{t}")
                    nc.vector.tensor_add(u[:], a[:], pp[:])
                    nc.vector.tensor_add(ob[:, zi : zi + ZT, :, 0, :], u[:], m0[:])
                    nc.vector.scalar_tensor_tensor(
                        ob[:, zi : zi + ZT, :, 1, :], q[:], 2.0, b[:], MULT, ADD
                    )
                    nc.vector.scalar_tensor_tensor(
                        ob[:, zi : zi + ZT, :, 2, :], pp[:], 4.0, a[:], MULT, ADD
                    )
                    t2 = ev_pool.tile([128, NT], BF16, name=f"t{t}", tag=f"t{t}")
                    nc.vector.scalar_tensor_tensor(t2[:], q[:], 8.0, b[:], MULT, ADD)
                    nc.vector.tensor_add(ob[:, zi : zi + ZT, :, 3, :], t2[:], m5[:])
                nc.sync.dma_start(out=y_out[:], in_=ob[:])

            if loop_n is not None:
                if unroll:
                    for _k in range(loop_n):
                        body()
                else:
                    with tc.For_i(0, loop_n, 1) as _i:
                        for _j in range(inner):
                            body(_i)
            else:
                body()

    nc.finalize()
    _elide_redundant_ldweights(nc)
    return nc


def _wtap(gw, kk, dz, dx):
    return gw[kk, :, :, dz, dx].T


def _transform_w(weight):
    w = np.asarray(weight, np.float32).reshape(COUT, CIN, K, K, K)
    gw = np.einsum("ky,oczyx->koczx", G, w)  # (6, O, C, 3z, 3x)
    wk = np.zeros((128, NK * NP_K, 128), np.float32)
    for kk in range(NK):
        for p, (dz, dx, lo, hi, kind) in enumerate(KPASSES):
            j = kk * NP_K + p
            if kind == "dual":  # (dz, dx) + (dz, dx+1)
                wk[0:64, j] = _wtap(gw, kk, dz, dx)
                wk[64:128, j] = _wtap(gw, kk, dz, dx + 1)
            elif kind == "lo":  # lower single (dz, dx)
                wk[0:64, j] = _wtap(gw, kk, dz, dx)
            else:  # upper single (dz, dx+1); lower rows stay zero
                wk[64:128, j] = _wtap(gw, kk, dz, dx + 1)
    return wk.astype(ml_dtypes.bfloat16)


def _make_in_maps(x, weight):
    wk = _transform_w(weight)
    x = np.asarray(x, np.float32)
    in_maps = []
    for c in range(N_CORES):
        b, zh = divmod(c, 2)
        z0 = zh * ZS
        xpad = np.zeros((CIN, PW, PW, PW), np.float32)
        xpad[:, 1:25, 1:25, 1:25] = x[b]
        win = xpad[:, z0 : z0 + NPL]  # (64, 14, 26, 26)
        # T[c, k, z, w, x] = sum_j BT[k, j] win[c, z, 4w+j, x]
        wmat = np.lib.stride_tricks.sliding_window_view(win, 6, axis=2)[:, :, ::4][
            :, :, :NW
        ]
        T = np.einsum("kj,czwxj->ckzwx", BT, wmat)  # (64, 6, 14, 6, 26)
        X = np.zeros((128, NK, NPL, NW, PW), np.float32)
        X[0:64] = T
        X[64:128, :, :, :, :-1] = T[:, :, :, :, 1:]  # +1x shift
        in_maps.append({"wk": wk, "xd": X.astype(ml_dtypes.bfloat16)})
    return in_maps


def _gather(results):
    out = np.empty((4, COUT, DHW, DHW, DHW), np.float32)
    for c in range(N_CORES):
        b, zh = divmod(c, 2)
        out[b, :, zh * ZS : (zh + 1) * ZS] = results[c]["y"].astype(np.float32)
    return out


def kernel(x, weight):
    from concourse.bass_utils import run_bass_kernel_spmd

    in_maps = _make_in_maps(x, weight)
    nc = _build_program()
    res = run_bass_kernel_spmd(nc, in_maps, list(range(N_CORES)))
    return _gather(res.results)


def _emulate_core(m):
    """Numpy model of one core incl. bf16 rounding of the AT chain."""
    X = np.asarray(m["xd"], np.float32)
    WK = np.asarray(m["wk"], np.float32)
    bf = lambda a: a.astype(ml_dtypes.bfloat16).astype(np.float32)
    y = np.zeros((128, ZS, DHW, DHW), np.float32)
    for zi in ZTILES:
        ps = np.zeros((NK, 128, NT), np.float32)
        for kk in range(NK):
            for p, (dz, dx, lo, hi, _k) in enumerate(KPASSES):
                j = kk * NP_K + p
                r = X[lo:hi, kk, zi + dz : zi + dz + ZT, 0:NW, dx : dx + 24]
                ps[kk] += WK[lo:hi, j].T @ r.reshape(hi - lo, -1)
        m0 = bf(ps[0])
        m5 = bf(ps[5])
        a = bf(ps[1] + ps[2])
        b_ = bf(ps[1] - ps[2])
        pp = bf(ps[3] + ps[4])
        q = bf(ps[3] - ps[4])
        u = bf(a + pp)
        rows = [bf(u + m0), bf(2 * q + b_), bf(4 * pp + a), bf(bf(8 * q + b_) + m5)]
        yi = np.stack([r.reshape(128, ZT, NW, 24) for r in rows], axis=3)
        y[:, zi : zi + ZT] = yi.reshape(128, ZT, 24, 24)
    return y


if __name__ == "__main__":
    import jax

    sys.path.insert(0, "/root/problem")
    import reference

    cpu = jax.devices("cpu")[0]
    with jax.default_device(cpu):
        inputs = {k: np.asarray(v) for k, v in reference.setup_inputs().items()}
        expected = np.asarray(
            reference.reference(**{k: jax.device_put(v, cpu) for k, v in inputs.items()})
        )
    in_maps = _make_in_maps(inputs["x"], inputs["weight"])
    y = _emulate_core(in_maps[0])
    exp = expected[0][:, 0:ZS]
    err = np.linalg.norm(y - exp) / np.linalg.norm(exp)
    print("emulated core0 rel err:", err)


# revision 28
# speedup vs baseline: 1.4194x; 1.3654x over previous
"""Winograd F(4,3)-y conv3d, v4: kk-major, chunked DMA, single X tensor.

Per-core work: 4 z-tiles x 3 planes, N=432 per matmul, 6 winograd
m-terms (kk) x 6 passes covering the 9 (dz,dx) taps as 3 x-duals
(128-contraction using the +1x-shifted upper half) + 3 singles (the
(1,2) single reads the upper half at dx=1 via tile_position (64,0)).

Key structure vs the 56.7us baseline:
- ONE HBM X tensor xd = [T; T(+1x)] (3.35MB, vs 7.7MB) DMA'd in 6
  per-kk chunks on the SP ring; W on the ACT ring. No on-chip copies.
- kk-major matmul order: all 4 z-tiles consume chunk kk right after it
  lands, so the PE starts ~4us in and chunk buffers free early -- the
  For_i hardware loop pipelines across iterations even with static
  tile buffers.
- Inverse transform fused into evacuation: ACT copies m0/m1/m3/m5 out
  of PSUM, DVE forms a/b/p/q with one PSUM operand each (HW limit:
  max one PSUM input per DVE op), scalar_tensor_tensor fuses the
  *2/*4/*8 scaled adds, intermediates bf16 (2x DVE rate).
"""

import sys

if "/opt/trn_rl_repo" not in sys.path:
    sys.path.insert(0, "/opt/trn_rl_repo")

import ml_dtypes
import numpy as np

CIN, COUT, K = 64, 128, 3
DHW = 24
ZS = 12  # z planes per core
NPL = 14  # input planes incl halo
PW = 26
NW = 6  # y window count (stride 4, size 6)
NK = 6  # winograd m-terms per window
N_CORES = 8
ZT = 3  # z planes per tile
ZTILES = (0, 3, 6, 9)
NT = ZT * NW * 24  # 432 cols per matmul

BT = np.array(
    [
        [4, 0, -5, 0, 1, 0],
        [0, -4, -4, 1, 1, 0],
        [0, 4, -4, -1, 1, 0],
        [0, -2, -1, 2, 1, 0],
        [0, 2, -1, -2, 1, 0],
        [0, 4, 0, -5, 0, 1],
    ],
    np.float32,
)
G = np.array(
    [
        [1 / 4, 0, 0],
        [-1 / 6, -1 / 6, -1 / 6],
        [-1 / 6, 1 / 6, -1 / 6],
        [1 / 24, 1 / 12, 1 / 6],
        [1 / 24, -1 / 12, 1 / 6],
        [0, 0, 1],
    ],
    np.float32,
)

# per-(kk,tile) passes: (dz, dx_ap, lo, hi). All passes read xd=[T; T(+1x)]:
# a [0:128] pass computes taps (dz, dx_ap) + (dz, dx_ap+1); a [0:64] pass
# the single lower tap (dz, dx_ap); a [64:128] pass the single upper tap
# (dz, dx_ap+1) via tile_position (64,0).
KPASSES = (
    (0, 0, 0, 128, "dual"),  # (0,0)+(0,1)
    (1, 0, 0, 128, "dual"),  # (1,0)+(1,1)
    (2, 0, 0, 128, "dual"),  # (2,0)+(2,1)
    (0, 2, 0, 64, "lo"),  # single (0,2)
    (1, 1, 0, 128, "up"),  # single (1,2): upper half only, zero lower weights
    (2, 2, 0, 64, "lo"),  # single (2,2)
)
NP_K = len(KPASSES)  # 6


def _elide_redundant_ldweights(nc):
    n_drop = 0
    for f in nc.m.functions:
        for b in f.blocks:
            last_key = None
            drop = []
            for inst in b.instructions:
                if type(inst).__name__ == "InstLdweights":
                    key = (str(inst.ins[0]), str(inst.perf_mode), str(inst.is_transpose))
                    si = inst.sync_info
                    clean = si is None or (len(si.on_wait) == 0 and len(si.on_update) == 0)
                    if key == last_key and clean:
                        drop.append(inst)
                    else:
                        last_key = key
            for inst in drop:
                b.instructions.remove(inst)
            n_drop += len(drop)
    return n_drop


def _build_program(loop_n=None, unroll=False, inner=1):
    import concourse.tile as tile
    from concourse import bacc, mybir

    BF16 = mybir.dt.bfloat16
    F32 = mybir.dt.float32
    MULT = mybir.AluOpType.mult
    ADD = mybir.AluOpType.add

    nc = bacc.Bacc("TRN2")
    xd_in = nc.declare_dram_parameter("xd", [128, NK, NPL, NW, PW], BF16, isOutput=False)
    wk_in = nc.declare_dram_parameter("wk", [128, NK * NP_K, 128], BF16, isOutput=False)
    y_out = nc.declare_dram_parameter("y", [128, ZS, DHW, DHW], BF16, isOutput=True)

    with tile.TileContext(nc) as tc:
        with (
            tc.tile_pool(name="xw", bufs=1) as xw_pool,
            tc.tile_pool(name="ps", bufs=8, space="PSUM") as ps_pool,
            tc.tile_pool(name="ev", bufs=2) as ev_pool,
            tc.tile_pool(name="ob", bufs=4) as ob_pool,
        ):

            def body(W):
                # 3 chunks of 2 kk each: few dma_starts (fixed cost ~2us each
                # on the ring) but the first matmuls still start early
                xdk = []
                for ck in range(3):
                    xd = xw_pool.tile(
                        [128, 2, NPL, NW, PW], BF16, name=f"xd{ck}", tag=f"xd{ck}"
                    )
                    nc.sync.dma_start(out=xd[:], in_=xd_in[:, 2 * ck : 2 * ck + 2])
                    xdk.append(xd)

                # psum accumulators, one bank per (tile, kk) group, evacuated
                # progressively so 8 banks suffice in kk-major order
                psq = {}
                evt = {}
                for kk in range(NK):
                    for t, zi in enumerate(ZTILES):
                        ps = ps_pool.tile([128, 512], F32, name="ps", tag="ps")
                        psq[(kk, t)] = ps
                        for p, (dz, dx, lo, hi, _k) in enumerate(KPASSES):
                            j = kk * NP_K + p
                            nc.tensor.matmul(
                                ps[:, :NT],
                                lhsT=W[lo:hi, j, :],
                                rhs=xdk[kk // 2][
                                    lo:hi, kk % 2, zi + dz : zi + dz + ZT, 0:NW,
                                    dx : dx + 24,
                                ],
                                start=(p == 0),
                                stop=(p == NP_K - 1),
                                skip_group_check=True,
                            )
                    # progressive evacuation: frees the two source banks per op
                    for t in range(4):
                        ps = psq[(kk, t)]

                        def ev(nm, _t=t, dt=BF16):
                            tl = ev_pool.tile(
                                [128, NT], dt, name=f"{nm}{_t}", tag=f"{nm}{_t}"
                            )
                            evt[(nm, _t)] = tl
                            return tl

                        # DVE may read only ONE PSUM operand per op, so odd
                        # m-terms go through an ACT f32 copy first
                        if kk == 0:
                            m0 = ev("m0")
                            nc.scalar.copy(m0[:], ps[:, :NT])
                        elif kk == 1:
                            m1 = ev("m1", dt=F32)
                            nc.scalar.copy(m1[:], ps[:, :NT])
                        elif kk == 2:
                            a = ev("a")
                            nc.vector.tensor_add(
                                a[:], evt[("m1", t)][:], ps[:, :NT]
                            )
                            b = ev("b")
                            nc.vector.tensor_sub(
                                b[:], evt[("m1", t)][:], ps[:, :NT]
                            )
                        elif kk == 3:
                            m3 = ev("m3", dt=F32)
                            nc.scalar.copy(m3[:], ps[:, :NT])
                        elif kk == 4:
                            pp = ev("p")
                            nc.vector.tensor_add(
                                pp[:], evt[("m3", t)][:], ps[:, :NT]
                            )
                            q = ev("q")
                            nc.vector.tensor_sub(
                                q[:], evt[("m3", t)][:], ps[:, :NT]
                            )
                        elif kk == 5:
                            m5 = ev("m5")
                            nc.scalar.copy(m5[:], ps[:, :NT])

                # final combine per tile into one merged output buffer,
                # then a single y store (one dma_start instead of four)
                ob = ob_pool.tile([128, ZS, NW, 4, 24], BF16, name="ob", tag="ob")
                for t, zi in enumerate(ZTILES):
                    m0, m5, a, b, pp, q = (
                        evt[(nm, t)] for nm in ("m0", "m5", "a", "b", "p", "q")
                    )
                    u = ev_pool.tile([128, NT], BF16, name=f"u{t}", tag=f"u{t}")
                    nc.vector.tensor_add(u[:], a[:], pp[:])
                    nc.vector.tensor_add(ob[:, zi : zi + ZT, :, 0, :], u[:], m0[:])
                    nc.vector.scalar_tensor_tensor(
                        ob[:, zi : zi + ZT, :, 1, :], q[:], 2.0, b[:], MULT, ADD
                    )
                    nc.vector.scalar_tensor_tensor(
                        ob[:, zi : zi + ZT, :, 2, :], pp[:], 4.0, a[:], MULT, ADD
                    )
                    t2 = ev_pool.tile([128, NT], BF16, name=f"t{t}", tag=f"t{t}")
                    nc.vector.scalar_tensor_tensor(t2[:], q[:], 8.0, b[:], MULT, ADD)
                    nc.vector.tensor_add(ob[:, zi : zi + ZT, :, 3, :], t2[:], m5[:])
                # y store on the idle SWDGE/Pool ring so it never head-of-line
                # blocks the next body's xd loads (SP) or W/copies (ACT)
                nc.gpsimd.dma_start(out=y_out[:], in_=ob[:])

            def block(n_bodies):
                # weights are loop-invariant: one load per block
                W = xw_pool.tile([128, NK * NP_K, 128], BF16, name="W", tag="W")
                nc.scalar.dma_start(out=W[:], in_=wk_in[:])
                for _ in range(n_bodies):
                    body(W)

            if loop_n is not None:
                if unroll:
                    for _k in range(loop_n):
                        block(1)
                else:
                    with tc.For_i(0, loop_n, 1) as _i:
                        block(inner)
            else:
                block(1)

    nc.finalize()
    _elide_redundant_ldweights(nc)
    return nc


def _wtap(gw, kk, dz, dx):
    return gw[kk, :, :, dz, dx].T


def _transform_w(weight):
    w = np.asarray(weight, np.float32).reshape(COUT, CIN, K, K, K)
    gw = np.einsum("ky,oczyx->koczx", G, w)  # (6, O, C, 3z, 3x)
    wk = np.zeros((128, NK * NP_K, 128), np.float32)
    for kk in range(NK):
        for p, (dz, dx, lo, hi, kind) in enumerate(KPASSES):
            j = kk * NP_K + p
            if kind == "dual":  # (dz, dx) + (dz, dx+1)
                wk[0:64, j] = _wtap(gw, kk, dz, dx)
                wk[64:128, j] = _wtap(gw, kk, dz, dx + 1)
            elif kind == "lo":  # lower single (dz, dx)
                wk[0:64, j] = _wtap(gw, kk, dz, dx)
            else:  # upper single (dz, dx+1); lower rows stay zero
                wk[64:128, j] = _wtap(gw, kk, dz, dx + 1)
    return wk.astype(ml_dtypes.bfloat16)


def _make_in_maps(x, weight):
    wk = _transform_w(weight)
    x = np.asarray(x, np.float32)
    in_maps = []
    for c in range(N_CORES):
        b, zh = divmod(c, 2)
        z0 = zh * ZS
        xpad = np.zeros((CIN, PW, PW, PW), np.float32)
        xpad[:, 1:25, 1:25, 1:25] = x[b]
        win = xpad[:, z0 : z0 + NPL]  # (64, 14, 26, 26)
        # T[c, k, z, w, x] = sum_j BT[k, j] win[c, z, 4w+j, x]
        wmat = np.lib.stride_tricks.sliding_window_view(win, 6, axis=2)[:, :, ::4][
            :, :, :NW
        ]
        T = np.einsum("kj,czwxj->ckzwx", BT, wmat)  # (64, 6, 14, 6, 26)
        X = np.zeros((128, NK, NPL, NW, PW), np.float32)
        X[0:64] = T
        X[64:128, :, :, :, :-1] = T[:, :, :, :, 1:]  # +1x shift
        in_maps.append({"wk": wk, "xd": X.astype(ml_dtypes.bfloat16)})
    return in_maps


def _gather(results):
    out = np.empty((4, COUT, DHW, DHW, DHW), np.float32)
    for c in range(N_CORES):
        b, zh = divmod(c, 2)
        out[b, :, zh * ZS : (zh + 1) * ZS] = results[c]["y"].astype(np.float32)
    return out


def kernel(x, weight):
    from concourse.bass_utils import run_bass_kernel_spmd

    in_maps = _make_in_maps(x, weight)
    nc = _build_program()
    res = run_bass_kernel_spmd(nc, in_maps, list(range(N_CORES)))
    return _gather(res.results)


def _emulate_core(m):
    """Numpy model of one core incl. bf16 rounding of the AT chain."""
    X = np.asarray(m["xd"], np.float32)
    WK = np.asarray(m["wk"], np.float32)
    bf = lambda a: a.astype(ml_dtypes.bfloat16).astype(np.float32)
    y = np.zeros((128, ZS, DHW, DHW), np.float32)
    for zi in ZTILES:
        ps = np.zeros((NK, 128, NT), np.float32)
        for kk in range(NK):
            for p, (dz, dx, lo, hi, _k) in enumerate(KPASSES):
                j = kk * NP_K + p
                r = X[lo:hi, kk, zi + dz : zi + dz + ZT, 0:NW, dx : dx + 24]
                ps[kk] += WK[lo:hi, j].T @ r.reshape(hi - lo, -1)
        m0 = bf(ps[0])
        m5 = bf(ps[5])
        a = bf(ps[1] + ps[2])
        b_ = bf(ps[1] - ps[2])
        pp = bf(ps[3] + ps[4])
        q = bf(ps[3] - ps[4])
        u = bf(a + pp)
        rows = [bf(u + m0), bf(2 * q + b_), bf(4 * pp + a), bf(bf(8 * q + b_) + m5)]
        yi = np.stack([r.reshape(128, ZT, NW, 24) for r in rows], axis=3)
        y[:, zi : zi + ZT] = yi.reshape(128, ZT, 24, 24)
    return y


if __name__ == "__main__":
    import jax

    sys.path.insert(0, "/root/problem")
    import reference

    cpu = jax.devices("cpu")[0]
    with jax.default_device(cpu):
        inputs = {k: np.asarray(v) for k, v in reference.setup_inputs().items()}
        expected = np.asarray(
            reference.reference(**{k: jax.device_put(v, cpu) for k, v in inputs.items()})
        )
    in_maps = _make_in_maps(inputs["x"], inputs["weight"])
    y = _emulate_core(in_maps[0])
    exp = expected[0][:, 0:ZS]
    err = np.linalg.norm(y - exp) / np.linalg.norm(exp)
    print("emulated core0 rel err:", err)


# revision 31
# speedup vs baseline: 1.5930x; 1.1223x over previous
"""Winograd F(4,3)-y conv3d, v4: kk-major, chunked DMA, single X tensor.

Per-core work: 4 z-tiles x 3 planes, N=432 per matmul, 6 winograd
m-terms (kk) x 6 passes covering the 9 (dz,dx) taps as 3 x-duals
(128-contraction using the +1x-shifted upper half) + 3 singles (the
(1,2) single reads the upper half at dx=1 via tile_position (64,0)).

Key structure vs the 56.7us baseline:
- ONE HBM X tensor xd = [T; T(+1x)] (3.35MB, vs 7.7MB) DMA'd in 6
  per-kk chunks on the SP ring; W on the ACT ring. No on-chip copies.
- kk-major matmul order: all 4 z-tiles consume chunk kk right after it
  lands, so the PE starts ~4us in and chunk buffers free early -- the
  For_i hardware loop pipelines across iterations even with static
  tile buffers.
- Inverse transform fused into evacuation: ACT copies m0/m1/m3/m5 out
  of PSUM, DVE forms a/b/p/q with one PSUM operand each (HW limit:
  max one PSUM input per DVE op), scalar_tensor_tensor fuses the
  *2/*4/*8 scaled adds, intermediates bf16 (2x DVE rate).
"""

import sys

if "/opt/trn_rl_repo" not in sys.path:
    sys.path.insert(0, "/opt/trn_rl_repo")

import ml_dtypes
import numpy as np

CIN, COUT, K = 64, 128, 3
DHW = 24
ZS = 12  # z planes per core
NPL = 14  # input planes incl halo
PW = 26
NW = 6  # y window count (stride 4, size 6)
NK = 6  # winograd m-terms per window
N_CORES = 8
ZT = 3  # z planes per tile
ZTILES = (0, 3, 6, 9)
NT = ZT * NW * 24  # 432 cols per matmul

BT = np.array(
    [
        [4, 0, -5, 0, 1, 0],
        [0, -4, -4, 1, 1, 0],
        [0, 4, -4, -1, 1, 0],
        [0, -2, -1, 2, 1, 0],
        [0, 2, -1, -2, 1, 0],
        [0, 4, 0, -5, 0, 1],
    ],
    np.float32,
)
G = np.array(
    [
        [1 / 4, 0, 0],
        [-1 / 6, -1 / 6, -1 / 6],
        [-1 / 6, 1 / 6, -1 / 6],
        [1 / 24, 1 / 12, 1 / 6],
        [1 / 24, -1 / 12, 1 / 6],
        [0, 0, 1],
    ],
    np.float32,
)

# per-(kk,tile) passes: (dz, dx_ap, lo, hi). All passes read xd=[T; T(+1x)]:
# a [0:128] pass computes taps (dz, dx_ap) + (dz, dx_ap+1); a [0:64] pass
# the single lower tap (dz, dx_ap); a [64:128] pass the single upper tap
# (dz, dx_ap+1) via tile_position (64,0).
KPASSES = (
    (0, 0, 0, 128, "dual"),  # (0,0)+(0,1)
    (1, 0, 0, 128, "dual"),  # (1,0)+(1,1)
    (2, 0, 0, 128, "dual"),  # (2,0)+(2,1)
    (0, 2, 0, 64, "lo"),  # single (0,2)
    (1, 1, 0, 128, "up"),  # single (1,2): upper half only, zero lower weights
    (2, 2, 0, 64, "lo"),  # single (2,2)
)
NP_K = len(KPASSES)  # 6


def _elide_redundant_ldweights(nc):
    n_drop = 0
    for f in nc.m.functions:
        for b in f.blocks:
            last_key = None
            drop = []
            for inst in b.instructions:
                if type(inst).__name__ == "InstLdweights":
                    key = (str(inst.ins[0]), str(inst.perf_mode), str(inst.is_transpose))
                    si = inst.sync_info
                    clean = si is None or (len(si.on_wait) == 0 and len(si.on_update) == 0)
                    if key == last_key and clean:
                        drop.append(inst)
                    else:
                        last_key = key
            for inst in drop:
                b.instructions.remove(inst)
            n_drop += len(drop)
    return n_drop


def _build_program(loop_n=None, unroll=False, inner=1, bodies=1):
    import concourse.tile as tile
    from concourse import bacc, mybir

    BF16 = mybir.dt.bfloat16
    F32 = mybir.dt.float32
    MULT = mybir.AluOpType.mult
    ADD = mybir.AluOpType.add

    nc = bacc.Bacc("TRN2")
    xd_in = nc.declare_dram_parameter("xd", [128, NK, NPL, NW, PW], BF16, isOutput=False)
    wk_in = nc.declare_dram_parameter("wk", [128, NK * NP_K, 128], BF16, isOutput=False)
    y_out = nc.declare_dram_parameter("y", [128, ZS, DHW, DHW], BF16, isOutput=True)

    with tile.TileContext(nc) as tc:
        with (
            tc.tile_pool(name="xw", bufs=1) as xw_pool,
            tc.tile_pool(name="ps", bufs=8, space="PSUM") as ps_pool,
            tc.tile_pool(name="ev", bufs=2) as ev_pool,
            tc.tile_pool(name="ob", bufs=4) as ob_pool,
        ):

            def body(W):
                # 3 chunks of 2 kk each: few dma_starts (fixed cost ~2us each
                # on the ring) but the first matmuls still start early
                xdk = []
                for ck in range(3):
                    xd = xw_pool.tile(
                        [128, 2, NPL, NW, PW], BF16, name=f"xd{ck}", tag=f"xd{ck}"
                    )
                    nc.sync.dma_start(out=xd[:], in_=xd_in[:, 2 * ck : 2 * ck + 2])
                    xdk.append(xd)

                # psum accumulators, one bank per (tile, kk) group, evacuated
                # progressively so 8 banks suffice in kk-major order
                psq = {}
                evt = {}
                for kk in range(NK):
                    for t in range(4):
                        psq[(kk, t)] = ps_pool.tile(
                            [128, 512], F32, name="ps", tag="ps"
                        )
                    # pass-major over the 4 z-tiles: consecutive matmuls share
                    # the same stationary weights, so 3 of every 4 LDWEIGHTS
                    # get elided (same-row-group LDW cannot hide behind a
                    # running matmul, so each unelided one costs ~100ns)
                    for p, (dz, dx, lo, hi, _k) in enumerate(KPASSES):
                        j = kk * NP_K + p
                        for t, zi in enumerate(ZTILES):
                            nc.tensor.matmul(
                                psq[(kk, t)][:, :NT],
                                lhsT=W[lo:hi, j, :],
                                rhs=xdk[kk // 2][
                                    lo:hi, kk % 2, zi + dz : zi + dz + ZT, 0:NW,
                                    dx : dx + 24,
                                ],
                                start=(p == 0),
                                stop=(p == NP_K - 1),
                                skip_group_check=True,
                            )
                    # progressive evacuation: frees the two source banks per op
                    for t in range(4):
                        ps = psq[(kk, t)]

                        def ev(nm, _t=t, dt=BF16):
                            tl = ev_pool.tile(
                                [128, NT], dt, name=f"{nm}{_t}", tag=f"{nm}{_t}"
                            )
                            evt[(nm, _t)] = tl
                            return tl

                        # DVE may read only ONE PSUM operand per op, so odd
                        # m-terms go through an ACT f32 copy first
                        if kk == 0:
                            m0 = ev("m0")
                            nc.scalar.copy(m0[:], ps[:, :NT])
                        elif kk == 1:
                            m1 = ev("m1", dt=F32)
                            nc.scalar.copy(m1[:], ps[:, :NT])
                        elif kk == 2:
                            a = ev("a")
                            nc.vector.tensor_add(
                                a[:], evt[("m1", t)][:], ps[:, :NT]
                            )
                            b = ev("b")
                            nc.vector.tensor_sub(
                                b[:], evt[("m1", t)][:], ps[:, :NT]
                            )
                        elif kk == 3:
                            m3 = ev("m3", dt=F32)
                            nc.scalar.copy(m3[:], ps[:, :NT])
                        elif kk == 4:
                            pp = ev("p")
                            nc.vector.tensor_add(
                                pp[:], evt[("m3", t)][:], ps[:, :NT]
                            )
                            q = ev("q")
                            nc.vector.tensor_sub(
                                q[:], evt[("m3", t)][:], ps[:, :NT]
                            )
                        elif kk == 5:
                            m5 = ev("m5")
                            nc.scalar.copy(m5[:], ps[:, :NT])

                # final combine per tile into one merged output buffer,
                # then a single y store (one dma_start instead of four)
                ob = ob_pool.tile([128, ZS, NW, 4, 24], BF16, name="ob", tag="ob")
                for t, zi in enumerate(ZTILES):
                    m0, m5, a, b, pp, q = (
                        evt[(nm, t)] for nm in ("m0", "m5", "a", "b", "p", "q")
                    )
                    u = ev_pool.tile([128, NT], BF16, name=f"u{t}", tag=f"u{t}")
                    nc.vector.tensor_add(u[:], a[:], pp[:])
                    nc.vector.tensor_add(ob[:, zi : zi + ZT, :, 0, :], u[:], m0[:])
                    nc.vector.scalar_tensor_tensor(
                        ob[:, zi : zi + ZT, :, 1, :], q[:], 2.0, b[:], MULT, ADD
                    )
                    nc.vector.scalar_tensor_tensor(
                        ob[:, zi : zi + ZT, :, 2, :], pp[:], 4.0, a[:], MULT, ADD
                    )
                    t2 = ev_pool.tile([128, NT], BF16, name=f"t{t}", tag=f"t{t}")
                    nc.vector.scalar_tensor_tensor(t2[:], q[:], 8.0, b[:], MULT, ADD)
                    nc.vector.tensor_add(ob[:, zi : zi + ZT, :, 3, :], t2[:], m5[:])
                # y store on the idle SWDGE/Pool ring so it never head-of-line
                # blocks the next body's xd loads (SP) or W/copies (ACT)
                nc.gpsimd.dma_start(out=y_out[:], in_=ob[:])

            def block(n_bodies):
                # weights are loop-invariant: one load per block
                W = xw_pool.tile([128, NK * NP_K, 128], BF16, name="W", tag="W")
                nc.scalar.dma_start(out=W[:], in_=wk_in[:])
                for _ in range(n_bodies):
                    body(W)

            if loop_n is not None:
                if unroll:
                    for _k in range(loop_n):
                        block(1)
                else:
                    with tc.For_i(0, loop_n, 1) as _i:
                        block(inner)
            else:
                block(bodies)

    nc.finalize()
    _elide_redundant_ldweights(nc)
    return nc


def _wtap(gw, kk, dz, dx):
    return gw[kk, :, :, dz, dx].T


def _transform_w(weight):
    w = np.asarray(weight, np.float32).reshape(COUT, CIN, K, K, K)
    gw = np.einsum("ky,oczyx->koczx", G, w)  # (6, O, C, 3z, 3x)
    wk = np.zeros((128, NK * NP_K, 128), np.float32)
    for kk in range(NK):
        for p, (dz, dx, lo, hi, kind) in enumerate(KPASSES):
            j = kk * NP_K + p
            if kind == "dual":  # (dz, dx) + (dz, dx+1)
                wk[0:64, j] = _wtap(gw, kk, dz, dx)
                wk[64:128, j] = _wtap(gw, kk, dz, dx + 1)
            elif kind == "lo":  # lower single (dz, dx)
                wk[0:64, j] = _wtap(gw, kk, dz, dx)
            else:  # upper single (dz, dx+1); lower rows stay zero
                wk[64:128, j] = _wtap(gw, kk, dz, dx + 1)
    return wk.astype(ml_dtypes.bfloat16)


def _make_in_maps(x, weight):
    wk = _transform_w(weight)
    x = np.asarray(x, np.float32)
    in_maps = []
    for c in range(N_CORES):
        b, zh = divmod(c, 2)
        z0 = zh * ZS
        xpad = np.zeros((CIN, PW, PW, PW), np.float32)
        xpad[:, 1:25, 1:25, 1:25] = x[b]
        win = xpad[:, z0 : z0 + NPL]  # (64, 14, 26, 26)
        # T[c, k, z, w, x] = sum_j BT[k, j] win[c, z, 4w+j, x]
        wmat = np.lib.stride_tricks.sliding_window_view(win, 6, axis=2)[:, :, ::4][
            :, :, :NW
        ]
        T = np.einsum("kj,czwxj->ckzwx", BT, wmat)  # (64, 6, 14, 6, 26)
        X = np.zeros((128, NK, NPL, NW, PW), np.float32)
        X[0:64] = T
        X[64:128, :, :, :, :-1] = T[:, :, :, :, 1:]  # +1x shift
        in_maps.append({"wk": wk, "xd": X.astype(ml_dtypes.bfloat16)})
    return in_maps


def _gather(results):
    out = np.empty((4, COUT, DHW, DHW, DHW), np.float32)
    for c in range(N_CORES):
        b, zh = divmod(c, 2)
        out[b, :, zh * ZS : (zh + 1) * ZS] = results[c]["y"].astype(np.float32)
    return out


def kernel(x, weight):
    from concourse.bass_utils import run_bass_kernel_spmd

    in_maps = _make_in_maps(x, weight)
    nc = _build_program()
    res = run_bass_kernel_spmd(nc, in_maps, list(range(N_CORES)))
    return _gather(res.results)


def _emulate_core(m):
    """Numpy model of one core incl. bf16 rounding of the AT chain."""
    X = np.asarray(m["xd"], np.float32)
    WK = np.asarray(m["wk"], np.float32)
    bf = lambda a: a.astype(ml_dtypes.bfloat16).astype(np.float32)
    y = np.zeros((128, ZS, DHW, DHW), np.float32)
    for zi in ZTILES:
        ps = np.zeros((NK, 128, NT), np.float32)
        for kk in range(NK):
            for p, (dz, dx, lo, hi, _k) in enumerate(KPASSES):
                j = kk * NP_K + p
                r = X[lo:hi, kk, zi + dz : zi + dz + ZT, 0:NW, dx : dx + 24]
                ps[kk] += WK[lo:hi, j].T @ r.reshape(hi - lo, -1)
        m0 = bf(ps[0])
        m5 = bf(ps[5])
        a = bf(ps[1] + ps[2])
        b_ = bf(ps[1] - ps[2])
        pp = bf(ps[3] + ps[4])
        q = bf(ps[3] - ps[4])
        u = bf(a + pp)
        rows = [bf(u + m0), bf(2 * q + b_), bf(4 * pp + a), bf(bf(8 * q + b_) + m5)]
        yi = np.stack([r.reshape(128, ZT, NW, 24) for r in rows], axis=3)
        y[:, zi : zi + ZT] = yi.reshape(128, ZT, 24, 24)
    return y


if __name__ == "__main__":
    import jax

    sys.path.insert(0, "/root/problem")
    import reference

    cpu = jax.devices("cpu")[0]
    with jax.default_device(cpu):
        inputs = {k: np.asarray(v) for k, v in reference.setup_inputs().items()}
        expected = np.asarray(
            reference.reference(**{k: jax.device_put(v, cpu) for k, v in inputs.items()})
        )
    in_maps = _make_in_maps(inputs["x"], inputs["weight"])
    y = _emulate_core(in_maps[0])
    exp = expected[0][:, 0:ZS]
    err = np.linalg.norm(y - exp) / np.linalg.norm(exp)
    print("emulated core0 rel err:", err)


# revision 39
# speedup vs baseline: 1.5936x; 1.0004x over previous
"""Winograd F(4,3)-y conv3d, v4: kk-major, chunked DMA, single X tensor.

Per-core work: 4 z-tiles x 3 planes, N=432 per matmul, 6 winograd
m-terms (kk) x 6 passes covering the 9 (dz,dx) taps as 3 x-duals
(128-contraction using the +1x-shifted upper half) + 3 singles (the
(1,2) single reads the upper half at dx=1 via tile_position (64,0)).

Key structure vs the 56.7us baseline:
- ONE HBM X tensor xd = [T; T(+1x)] (3.35MB, vs 7.7MB) DMA'd in 6
  per-kk chunks on the SP ring; W on the ACT ring. No on-chip copies.
- kk-major matmul order: all 4 z-tiles consume chunk kk right after it
  lands, so the PE starts ~4us in and chunk buffers free early -- the
  For_i hardware loop pipelines across iterations even with static
  tile buffers.
- Inverse transform fused into evacuation: ACT copies m0/m1/m3/m5 out
  of PSUM, DVE forms a/b/p/q with one PSUM operand each (HW limit:
  max one PSUM input per DVE op), scalar_tensor_tensor fuses the
  *2/*4/*8 scaled adds, intermediates bf16 (2x DVE rate).
"""

import sys

if "/opt/trn_rl_repo" not in sys.path:
    sys.path.insert(0, "/opt/trn_rl_repo")

import ml_dtypes
import numpy as np

CIN, COUT, K = 64, 128, 3
DHW = 24
ZS = 12  # z planes per core
NPL = 14  # input planes incl halo
PW = 26
NW = 6  # y window count (stride 4, size 6)
NK = 6  # winograd m-terms per window
N_CORES = 8
ZT = 3  # z planes per tile
ZTILES = (0, 3, 6, 9)
NT = ZT * NW * 24  # 432 cols per matmul

BT = np.array(
    [
        [4, 0, -5, 0, 1, 0],
        [0, -4, -4, 1, 1, 0],
        [0, 4, -4, -1, 1, 0],
        [0, -2, -1, 2, 1, 0],
        [0, 2, -1, -2, 1, 0],
        [0, 4, 0, -5, 0, 1],
    ],
    np.float32,
)
G = np.array(
    [
        [1 / 4, 0, 0],
        [-1 / 6, -1 / 6, -1 / 6],
        [-1 / 6, 1 / 6, -1 / 6],
        [1 / 24, 1 / 12, 1 / 6],
        [1 / 24, -1 / 12, 1 / 6],
        [0, 0, 1],
    ],
    np.float32,
)

# per-(kk,tile) passes: (dz, dx_ap, lo, hi). All passes read xd=[T; T(+1x)]:
# a [0:128] pass computes taps (dz, dx_ap) + (dz, dx_ap+1); a [0:64] pass
# the single lower tap (dz, dx_ap); a [64:128] pass the single upper tap
# (dz, dx_ap+1) via tile_position (64,0).
KPASSES = (
    (0, 0, 0, 128, "dual"),  # (0,0)+(0,1)        from xd
    (1, 0, 0, 128, "dual"),  # (1,0)+(1,1)        from xd
    (2, 0, 0, 128, "dual"),  # (2,0)+(2,1)        from xd
    (2, 2, 0, 64, "lo"),  # single (2,2)          from xd
    (0, 2, 0, 128, "zdual"),  # (0,2)+(1,2)       from xe = [T; T(+1z)]
)
NP_K = len(KPASSES)  # 5


def _elide_redundant_ldweights(nc):
    n_drop = 0
    for f in nc.m.functions:
        for b in f.blocks:
            last_key = None
            drop = []
            for inst in b.instructions:
                if type(inst).__name__ == "InstLdweights":
                    key = (str(inst.ins[0]), str(inst.perf_mode), str(inst.is_transpose))
                    si = inst.sync_info
                    clean = si is None or (len(si.on_wait) == 0 and len(si.on_update) == 0)
                    if key == last_key and clean:
                        drop.append(inst)
                    else:
                        last_key = key
            for inst in drop:
                b.instructions.remove(inst)
            n_drop += len(drop)
    return n_drop


def _build_program(loop_n=None, unroll=False, inner=1, bodies=1):
    import concourse.tile as tile
    from concourse import bacc, mybir

    BF16 = mybir.dt.bfloat16
    F32 = mybir.dt.float32
    MULT = mybir.AluOpType.mult
    ADD = mybir.AluOpType.add

    nc = bacc.Bacc("TRN2")
    xd_in = nc.declare_dram_parameter("xd", [128, NK, NPL, NW, PW], BF16, isOutput=False)
    xe_in = nc.declare_dram_parameter("xe", [128, NK, NPL, NW, PW], BF16, isOutput=False)
    wk_in = nc.declare_dram_parameter("wk", [128, NK * NP_K, 128], BF16, isOutput=False)
    y_out = nc.declare_dram_parameter("y", [128, ZS, DHW, DHW], BF16, isOutput=True)

    with tile.TileContext(nc) as tc:
        with (
            tc.tile_pool(name="xw", bufs=1) as xw_pool,
            tc.tile_pool(name="ps", bufs=8, space="PSUM") as ps_pool,
            tc.tile_pool(name="ev", bufs=1) as ev_pool,
            tc.tile_pool(name="ob", bufs=2) as ob_pool,
        ):

            def body(W):
                # 3 chunks of 2 kk each: few dma_starts (fixed cost ~2us each
                # on the ring) but the first matmuls still start early
                xdk = []
                for ck in range(3):
                    xd = xw_pool.tile(
                        [128, 2, NPL, NW, PW], BF16, name=f"xd{ck}", tag=f"xd{ck}"
                    )
                    nc.sync.dma_start(out=xd[:], in_=xd_in[:, 2 * ck : 2 * ck + 2])
                    xdk.append(xd)
                # +1z variant for the z-dual pass rides the ACT ring
                xe = xw_pool.tile([128, NK, NPL, NW, PW], BF16, name="xe", tag="xe")
                nc.scalar.dma_start(out=xe[:], in_=xe_in[:])

                # psum accumulators, one bank per (tile, kk) group, evacuated
                # progressively so 8 banks suffice in kk-major order
                psq = {}
                evt = {}
                for kk in range(NK):
                    for t in range(4):
                        psq[(kk, t)] = ps_pool.tile(
                            [128, 512], F32, name="ps", tag="ps"
                        )
                    # pass-major over the 4 z-tiles: consecutive matmuls share
                    # the same stationary weights, so 3 of every 4 LDWEIGHTS
                    # get elided (same-row-group LDW cannot hide behind a
                    # running matmul, so each unelided one costs ~100ns)
                    for p, (dz, dx, lo, hi, kind) in enumerate(KPASSES):
                        j = kk * NP_K + p
                        for t, zi in enumerate(ZTILES):
                            if kind == "zdual":
                                rhs = xe[
                                    lo:hi, kk, zi + dz : zi + dz + ZT, 0:NW,
                                    dx : dx + 24,
                                ]
                            else:
                                rhs = xdk[kk // 2][
                                    lo:hi, kk % 2, zi + dz : zi + dz + ZT, 0:NW,
                                    dx : dx + 24,
                                ]
                            nc.tensor.matmul(
                                psq[(kk, t)][:, :NT],
                                lhsT=W[lo:hi, j, :],
                                rhs=rhs,
                                start=(p == 0),
                                stop=(p == NP_K - 1),
                                skip_group_check=True,
                            )
                    # progressive evacuation: frees the two source banks per op
                    for t in range(4):
                        ps = psq[(kk, t)]

                        def ev(nm, _t=t, dt=BF16):
                            tl = ev_pool.tile(
                                [128, NT], dt, name=f"{nm}{_t}", tag=f"{nm}{_t}"
                            )
                            evt[(nm, _t)] = tl
                            return tl

                        # DVE may read only ONE PSUM operand per op, so odd
                        # m-terms go through an ACT f32 copy first
                        if kk == 0:
                            m0 = ev("m0")
                            nc.scalar.copy(m0[:], ps[:, :NT])
                        elif kk == 1:
                            m1 = ev("m1", dt=F32)
                            nc.scalar.copy(m1[:], ps[:, :NT])
                        elif kk == 2:
                            a = ev("a")
                            nc.vector.tensor_add(
                                a[:], evt[("m1", t)][:], ps[:, :NT]
                            )
                            b = ev("b")
                            nc.vector.tensor_sub(
                                b[:], evt[("m1", t)][:], ps[:, :NT]
                            )
                        elif kk == 3:
                            m3 = ev("m3", dt=F32)
                            nc.scalar.copy(m3[:], ps[:, :NT])
                        elif kk == 4:
                            pp = ev("p")
                            nc.vector.tensor_add(
                                pp[:], evt[("m3", t)][:], ps[:, :NT]
                            )
                            q = ev("q")
                            nc.vector.tensor_sub(
                                q[:], evt[("m3", t)][:], ps[:, :NT]
                            )
                        elif kk == 5:
                            m5 = ev("m5")
                            nc.scalar.copy(m5[:], ps[:, :NT])

                # final combine per tile into one merged output buffer,
                # then a single y store (one dma_start instead of four)
                ob = ob_pool.tile([128, ZS, NW, 4, 24], BF16, name="ob", tag="ob")
                for t, zi in enumerate(ZTILES):
                    m0, m5, a, b, pp, q = (
                        evt[(nm, t)] for nm in ("m0", "m5", "a", "b", "p", "q")
                    )
                    u = ev_pool.tile([128, NT], BF16, name=f"u{t}", tag=f"u{t}")
                    nc.vector.tensor_add(u[:], a[:], pp[:])
                    nc.vector.tensor_add(ob[:, zi : zi + ZT, :, 0, :], u[:], m0[:])
                    nc.vector.scalar_tensor_tensor(
                        ob[:, zi : zi + ZT, :, 1, :], q[:], 2.0, b[:], MULT, ADD
                    )
                    nc.vector.scalar_tensor_tensor(
                        ob[:, zi : zi + ZT, :, 2, :], pp[:], 4.0, a[:], MULT, ADD
                    )
                    t2 = ev_pool.tile([128, NT], BF16, name=f"t{t}", tag=f"t{t}")
                    nc.vector.scalar_tensor_tensor(t2[:], q[:], 8.0, b[:], MULT, ADD)
                    nc.vector.tensor_add(ob[:, zi : zi + ZT, :, 3, :], t2[:], m5[:])
                # y store on the idle SWDGE/Pool ring so it never head-of-line
                # blocks the next body's xd loads (SP) or W/copies (ACT)
                nc.gpsimd.dma_start(out=y_out[:], in_=ob[:])

            def block(n_bodies):
                # weights are loop-invariant: one load per block
                W = xw_pool.tile([128, NK * NP_K, 128], BF16, name="W", tag="W")
                nc.scalar.dma_start(out=W[:], in_=wk_in[:])
                for _ in range(n_bodies):
                    body(W)

            if loop_n is not None:
                if unroll:
                    for _k in range(loop_n):
                        block(1)
                else:
                    with tc.For_i(0, loop_n, 1) as _i:
                        block(inner)
            else:
                block(bodies)

    nc.finalize()
    _elide_redundant_ldweights(nc)
    return nc


def _wtap(gw, kk, dz, dx):
    return gw[kk, :, :, dz, dx].T


def _transform_w(weight):
    w = np.asarray(weight, np.float32).reshape(COUT, CIN, K, K, K)
    gw = np.einsum("ky,oczyx->koczx", G, w)  # (6, O, C, 3z, 3x)
    wk = np.zeros((128, NK * NP_K, 128), np.float32)
    for kk in range(NK):
        for p, (dz, dx, lo, hi, kind) in enumerate(KPASSES):
            j = kk * NP_K + p
            if kind == "dual":  # (dz, dx) + (dz, dx+1)
                wk[0:64, j] = _wtap(gw, kk, dz, dx)
                wk[64:128, j] = _wtap(gw, kk, dz, dx + 1)
            elif kind == "lo":  # lower single (dz, dx)
                wk[0:64, j] = _wtap(gw, kk, dz, dx)
            else:  # zdual: (dz, dx) + (dz+1, dx) via the +1z upper half
                wk[0:64, j] = _wtap(gw, kk, dz, dx)
                wk[64:128, j] = _wtap(gw, kk, dz + 1, dx)
    return wk.astype(ml_dtypes.bfloat16)


def _make_in_maps(x, weight):
    wk = _transform_w(weight)
    x = np.asarray(x, np.float32)
    in_maps = []
    for c in range(N_CORES):
        b, zh = divmod(c, 2)
        z0 = zh * ZS
        xpad = np.zeros((CIN, PW, PW, PW), np.float32)
        xpad[:, 1:25, 1:25, 1:25] = x[b]
        win = xpad[:, z0 : z0 + NPL]  # (64, 14, 26, 26)
        # T[c, k, z, w, x] = sum_j BT[k, j] win[c, z, 4w+j, x]
        wmat = np.lib.stride_tricks.sliding_window_view(win, 6, axis=2)[:, :, ::4][
            :, :, :NW
        ]
        T = np.einsum("kj,czwxj->ckzwx", BT, wmat)  # (64, 6, 14, 6, 26)
        X = np.zeros((128, NK, NPL, NW, PW), np.float32)
        X[0:64] = T
        X[64:128, :, :, :, :-1] = T[:, :, :, :, 1:]  # +1x shift
        XE = np.zeros((128, NK, NPL, NW, PW), np.float32)
        XE[0:64] = T
        XE[64:128, :, :-1] = T[:, :, 1:]  # +1z shift
        in_maps.append(
            {
                "wk": wk,
                "xd": X.astype(ml_dtypes.bfloat16),
                "xe": XE.astype(ml_dtypes.bfloat16),
            }
        )
    return in_maps


def _gather(results):
    out = np.empty((4, COUT, DHW, DHW, DHW), np.float32)
    for c in range(N_CORES):
        b, zh = divmod(c, 2)
        out[b, :, zh * ZS : (zh + 1) * ZS] = results[c]["y"].astype(np.float32)
    return out


def kernel(x, weight):
    from concourse.bass_utils import run_bass_kernel_spmd

    in_maps = _make_in_maps(x, weight)
    nc = _build_program()
    res = run_bass_kernel_spmd(nc, in_maps, list(range(N_CORES)))
    return _gather(res.results)


def _emulate_core(m):
    """Numpy model of one core incl. bf16 rounding of the AT chain."""
    X = np.asarray(m["xd"], np.float32)
    XE = np.asarray(m["xe"], np.float32)
    WK = np.asarray(m["wk"], np.float32)
    bf = lambda a: a.astype(ml_dtypes.bfloat16).astype(np.float32)
    y = np.zeros((128, ZS, DHW, DHW), np.float32)
    for zi in ZTILES:
        ps = np.zeros((NK, 128, NT), np.float32)
        for kk in range(NK):
            for p, (dz, dx, lo, hi, kind) in enumerate(KPASSES):
                j = kk * NP_K + p
                XX = XE if kind == "zdual" else X
                r = XX[lo:hi, kk, zi + dz : zi + dz + ZT, 0:NW, dx : dx + 24]
                ps[kk] += WK[lo:hi, j].T @ r.reshape(hi - lo, -1)
        m0 = bf(ps[0])
        m5 = bf(ps[5])
        a = bf(ps[1] + ps[2])
        b_ = bf(ps[1] - ps[2])
        pp = bf(ps[3] + ps[4])
        q = bf(ps[3] - ps[4])
        u = bf(a + pp)
        rows = [bf(u + m0), bf(2 * q + b_), bf(4 * pp + a), bf(bf(8 * q + b_) + m5)]
        yi = np.stack([r.reshape(128, ZT, NW, 24) for r in rows], axis=3)
        y[:, zi : zi + ZT] = yi.reshape(128, ZT, 24, 24)
    return y


if __name__ == "__main__":
    import jax

    sys.path.insert(0, "/root/problem")
    import reference

    cpu = jax.devices("cpu")[0]
    with jax.default_device(cpu):
        inputs = {k: np.asarray(v) for k, v in reference.setup_inputs().items()}
        expected = np.asarray(
            reference.reference(**{k: jax.device_put(v, cpu) for k, v in inputs.items()})
        )
    in_maps = _make_in_maps(inputs["x"], inputs["weight"])
    y = _emulate_core(in_maps[0])
    exp = expected[0][:, 0:ZS]
    err = np.linalg.norm(y - exp) / np.linalg.norm(exp)
    print("emulated core0 rel err:", err)


# revision 41
# speedup vs baseline: 1.6174x; 1.0149x over previous
"""Winograd F(4,3)-y conv3d. Measured ~39.6us/conv per core (baseline
56.7us on the same min-slope harness; quoted 68.9us previously).

Measured decomposition of the 39.6us (HW ablations, inner=16):
DMA alone ~22.9us/conv (8.6MB at ~375GB/s = the HBM limit), compute
pipeline alone ~34.8us (120 MMs at ~210-260ns effective incl. issue/
sem overhead + evac latency fringes); input DMA mostly hides behind
compute, exposing only ~5us. Further MM-count cuts (144->120) and
byte cuts (xe on/off) trade 1:1 and do not move the total.

Per-core work: 4 z-tiles x 3 planes, N=432 per matmul (one PSUM bank),
6 winograd m-terms (kk) x 5 passes packing the 9 (dz,dx) taps as
3 x-duals + 1 single (from xd=[T; T(+1x)]) + 1 z-dual (from
xe=[T; T(+1z)]) -- 120 matmuls/conv.

What mattered on real HW (sim cost model was misleading):
- For_i carries an all-engine barrier + sem reset per iteration, so
  bodies are unrolled 8x inside the loop (`inner`) and weights load
  once per block; per-conv time is the marginal slope.
- HWDGE DMAs execute FIFO per issuing ring: y-store rides the idle
  SWDGE/Pool ring and xe the ACT ring so nothing head-of-line blocks
  the next body's xd chunks on the SP ring. On-chip SBUF->SBUF shift
  copies (tried first) were far slower than shipping both host-shifted
  tensors.
- Pass-major matmul order across the 4 z-tiles: consecutive MMs share
  stationary weights, eliding 3/4 LDWEIGHTS (same-row-group LDW can't
  hide behind a running matmul).
- Inverse transform fused into evacuation: ACT copies m0/m1/m3/m5 out
  of PSUM, DVE forms a/b/p/q with one PSUM operand each (HW limit:
  max one PSUM input per DVE op), scalar_tensor_tensor fuses the
  *2/*4/*8 scaled adds, intermediates bf16 (2x DVE rate). All of this
  overlaps the PE stream completely (measured: removing it changes
  nothing).
"""

import sys

if "/opt/trn_rl_repo" not in sys.path:
    sys.path.insert(0, "/opt/trn_rl_repo")

import ml_dtypes
import numpy as np

CIN, COUT, K = 64, 128, 3
DHW = 24
ZS = 12  # z planes per core
NPL = 14  # input planes incl halo
PW = 26
NW = 6  # y window count (stride 4, size 6)
NK = 6  # winograd m-terms per window
N_CORES = 8
ZT = 3  # z planes per tile
ZTILES = (0, 3, 6, 9)
NT = ZT * NW * 24  # 432 cols per matmul

BT = np.array(
    [
        [4, 0, -5, 0, 1, 0],
        [0, -4, -4, 1, 1, 0],
        [0, 4, -4, -1, 1, 0],
        [0, -2, -1, 2, 1, 0],
        [0, 2, -1, -2, 1, 0],
        [0, 4, 0, -5, 0, 1],
    ],
    np.float32,
)
G = np.array(
    [
        [1 / 4, 0, 0],
        [-1 / 6, -1 / 6, -1 / 6],
        [-1 / 6, 1 / 6, -1 / 6],
        [1 / 24, 1 / 12, 1 / 6],
        [1 / 24, -1 / 12, 1 / 6],
        [0, 0, 1],
    ],
    np.float32,
)

# per-(kk,tile) passes: (dz, dx_ap, lo, hi). All passes read xd=[T; T(+1x)]:
# a [0:128] pass computes taps (dz, dx_ap) + (dz, dx_ap+1); a [0:64] pass
# the single lower tap (dz, dx_ap); a [64:128] pass the single upper tap
# (dz, dx_ap+1) via tile_position (64,0).
KPASSES = (
    (0, 0, 0, 128, "dual"),  # (0,0)+(0,1)        from xd
    (1, 0, 0, 128, "dual"),  # (1,0)+(1,1)        from xd
    (2, 0, 0, 128, "dual"),  # (2,0)+(2,1)        from xd
    (2, 2, 0, 64, "lo"),  # single (2,2)          from xd
    (0, 2, 0, 128, "zdual"),  # (0,2)+(1,2)       from xe = [T; T(+1z)]
)
NP_K = len(KPASSES)  # 5


def _elide_redundant_ldweights(nc):
    n_drop = 0
    for f in nc.m.functions:
        for b in f.blocks:
            last_key = None
            drop = []
            for inst in b.instructions:
                if type(inst).__name__ == "InstLdweights":
                    key = (str(inst.ins[0]), str(inst.perf_mode), str(inst.is_transpose))
                    si = inst.sync_info
                    clean = si is None or (len(si.on_wait) == 0 and len(si.on_update) == 0)
                    if key == last_key and clean:
                        drop.append(inst)
                    else:
                        last_key = key
            for inst in drop:
                b.instructions.remove(inst)
            n_drop += len(drop)
    return n_drop


def _build_program(loop_n=None, unroll=False, inner=1, bodies=1):
    import concourse.tile as tile
    from concourse import bacc, mybir

    BF16 = mybir.dt.bfloat16
    F32 = mybir.dt.float32
    MULT = mybir.AluOpType.mult
    ADD = mybir.AluOpType.add

    nc = bacc.Bacc("TRN2")
    xd_in = nc.declare_dram_parameter("xd", [128, NK, NPL, NW, PW], BF16, isOutput=False)
    xe_in = nc.declare_dram_parameter("xe", [128, NK, NPL, NW, PW], BF16, isOutput=False)
    wk_in = nc.declare_dram_parameter("wk", [128, NK * NP_K, 128], BF16, isOutput=False)
    y_out = nc.declare_dram_parameter("y", [128, ZS, DHW, DHW], BF16, isOutput=True)

    with tile.TileContext(nc) as tc:
        with (
            tc.tile_pool(name="xw", bufs=1) as xw_pool,
            tc.tile_pool(name="ps", bufs=8, space="PSUM") as ps_pool,
            tc.tile_pool(name="ev", bufs=1) as ev_pool,
            tc.tile_pool(name="ob", bufs=2) as ob_pool,
        ):

            def body(W):
                # 3 chunks of 2 kk each: few dma_starts (fixed cost ~2us each
                # on the ring) but the first matmuls still start early
                xdk = []
                for ck in range(3):
                    xd = xw_pool.tile(
                        [128, 2, NPL, NW, PW], BF16, name=f"xd{ck}", tag=f"xd{ck}"
                    )
                    nc.sync.dma_start(out=xd[:], in_=xd_in[:, 2 * ck : 2 * ck + 2])
                    xdk.append(xd)
                # +1z variant for the z-dual pass rides the ACT ring
                xe = xw_pool.tile([128, NK, NPL, NW, PW], BF16, name="xe", tag="xe")
                nc.scalar.dma_start(out=xe[:], in_=xe_in[:])

                # psum accumulators, one bank per (tile, kk) group, evacuated
                # progressively so 8 banks suffice in kk-major order
                psq = {}
                evt = {}
                for kk in range(NK):
                    for t in range(4):
                        psq[(kk, t)] = ps_pool.tile(
                            [128, 512], F32, name="ps", tag="ps"
                        )
                    # pass-major over the 4 z-tiles: consecutive matmuls share
                    # the same stationary weights, so 3 of every 4 LDWEIGHTS
                    # get elided (same-row-group LDW cannot hide behind a
                    # running matmul, so each unelided one costs ~100ns)
                    for p, (dz, dx, lo, hi, kind) in enumerate(KPASSES):
                        j = kk * NP_K + p
                        for t, zi in enumerate(ZTILES):
                            if kind == "zdual":
                                rhs = xe[
                                    lo:hi, kk, zi + dz : zi + dz + ZT, 0:NW,
                                    dx : dx + 24,
                                ]
                            else:
                                rhs = xdk[kk // 2][
                                    lo:hi, kk % 2, zi + dz : zi + dz + ZT, 0:NW,
                                    dx : dx + 24,
                                ]
                            nc.tensor.matmul(
                                psq[(kk, t)][:, :NT],
                                lhsT=W[lo:hi, j, :],
                                rhs=rhs,
                                start=(p == 0),
                                stop=(p == NP_K - 1),
                                skip_group_check=True,
                            )
                    # progressive evacuation: frees the two source banks per op
                    for t in range(4):
                        ps = psq[(kk, t)]

                        def ev(nm, _t=t, dt=BF16):
                            tl = ev_pool.tile(
                                [128, NT], dt, name=f"{nm}{_t}", tag=f"{nm}{_t}"
                            )
                            evt[(nm, _t)] = tl
                            return tl

                        # DVE may read only ONE PSUM operand per op, so odd
                        # m-terms go through an ACT f32 copy first
                        if kk == 0:
                            m0 = ev("m0")
                            nc.scalar.copy(m0[:], ps[:, :NT])
                        elif kk == 1:
                            m1 = ev("m1", dt=F32)
                            nc.scalar.copy(m1[:], ps[:, :NT])
                        elif kk == 2:
                            a = ev("a")
                            nc.vector.tensor_add(
                                a[:], evt[("m1", t)][:], ps[:, :NT]
                            )
                            b = ev("b")
                            nc.vector.tensor_sub(
                                b[:], evt[("m1", t)][:], ps[:, :NT]
                            )
                        elif kk == 3:
                            m3 = ev("m3", dt=F32)
                            nc.scalar.copy(m3[:], ps[:, :NT])
                        elif kk == 4:
                            pp = ev("p")
                            nc.vector.tensor_add(
                                pp[:], evt[("m3", t)][:], ps[:, :NT]
                            )
                            q = ev("q")
                            nc.vector.tensor_sub(
                                q[:], evt[("m3", t)][:], ps[:, :NT]
                            )
                        elif kk == 5:
                            m5 = ev("m5")
                            nc.scalar.copy(m5[:], ps[:, :NT])

                # final combine per tile into one merged output buffer,
                # then a single y store (one dma_start instead of four)
                ob = ob_pool.tile([128, ZS, NW, 4, 24], BF16, name="ob", tag="ob")
                for t, zi in enumerate(ZTILES):
                    m0, m5, a, b, pp, q = (
                        evt[(nm, t)] for nm in ("m0", "m5", "a", "b", "p", "q")
                    )
                    u = ev_pool.tile([128, NT], BF16, name=f"u{t}", tag=f"u{t}")
                    nc.vector.tensor_add(u[:], a[:], pp[:])
                    nc.vector.tensor_add(ob[:, zi : zi + ZT, :, 0, :], u[:], m0[:])
                    nc.vector.scalar_tensor_tensor(
                        ob[:, zi : zi + ZT, :, 1, :], q[:], 2.0, b[:], MULT, ADD
                    )
                    nc.vector.scalar_tensor_tensor(
                        ob[:, zi : zi + ZT, :, 2, :], pp[:], 4.0, a[:], MULT, ADD
                    )
                    t2 = ev_pool.tile([128, NT], BF16, name=f"t{t}", tag=f"t{t}")
                    nc.vector.scalar_tensor_tensor(t2[:], q[:], 8.0, b[:], MULT, ADD)
                    nc.vector.tensor_add(ob[:, zi : zi + ZT, :, 3, :], t2[:], m5[:])
                # y store on the idle SWDGE/Pool ring so it never head-of-line
                # blocks the next body's xd loads (SP) or W/copies (ACT)
                nc.gpsimd.dma_start(out=y_out[:], in_=ob[:])

            def block(n_bodies):
                # weights are loop-invariant: one load per block
                W = xw_pool.tile([128, NK * NP_K, 128], BF16, name="W", tag="W")
                nc.scalar.dma_start(out=W[:], in_=wk_in[:])
                for _ in range(n_bodies):
                    body(W)

            if loop_n is not None:
                if unroll:
                    for _k in range(loop_n):
                        block(1)
                else:
                    with tc.For_i(0, loop_n, 1) as _i:
                        block(inner)
            else:
                block(bodies)

    nc.finalize()
    _elide_redundant_ldweights(nc)
    return nc


def _wtap(gw, kk, dz, dx):
    return gw[kk, :, :, dz, dx].T


def _transform_w(weight):
    w = np.asarray(weight, np.float32).reshape(COUT, CIN, K, K, K)
    gw = np.einsum("ky,oczyx->koczx", G, w)  # (6, O, C, 3z, 3x)
    wk = np.zeros((128, NK * NP_K, 128), np.float32)
    for kk in range(NK):
        for p, (dz, dx, lo, hi, kind) in enumerate(KPASSES):
            j = kk * NP_K + p
            if kind == "dual":  # (dz, dx) + (dz, dx+1)
                wk[0:64, j] = _wtap(gw, kk, dz, dx)
                wk[64:128, j] = _wtap(gw, kk, dz, dx + 1)
            elif kind == "lo":  # lower single (dz, dx)
                wk[0:64, j] = _wtap(gw, kk, dz, dx)
            else:  # zdual: (dz, dx) + (dz+1, dx) via the +1z upper half
                wk[0:64, j] = _wtap(gw, kk, dz, dx)
                wk[64:128, j] = _wtap(gw, kk, dz + 1, dx)
    return wk.astype(ml_dtypes.bfloat16)


def _make_in_maps(x, weight):
    wk = _transform_w(weight)
    x = np.asarray(x, np.float32)
    in_maps = []
    for c in range(N_CORES):
        b, zh = divmod(c, 2)
        z0 = zh * ZS
        xpad = np.zeros((CIN, PW, PW, PW), np.float32)
        xpad[:, 1:25, 1:25, 1:25] = x[b]
        win = xpad[:, z0 : z0 + NPL]  # (64, 14, 26, 26)
        # T[c, k, z, w, x] = sum_j BT[k, j] win[c, z, 4w+j, x]
        wmat = np.lib.stride_tricks.sliding_window_view(win, 6, axis=2)[:, :, ::4][
            :, :, :NW
        ]
        T = np.einsum("kj,czwxj->ckzwx", BT, wmat)  # (64, 6, 14, 6, 26)
        X = np.zeros((128, NK, NPL, NW, PW), np.float32)
        X[0:64] = T
        X[64:128, :, :, :, :-1] = T[:, :, :, :, 1:]  # +1x shift
        XE = np.zeros((128, NK, NPL, NW, PW), np.float32)
        XE[0:64] = T
        XE[64:128, :, :-1] = T[:, :, 1:]  # +1z shift
        in_maps.append(
            {
                "wk": wk,
                "xd": X.astype(ml_dtypes.bfloat16),
                "xe": XE.astype(ml_dtypes.bfloat16),
            }
        )
    return in_maps


def _gather(results):
    out = np.empty((4, COUT, DHW, DHW, DHW), np.float32)
    for c in range(N_CORES):
        b, zh = divmod(c, 2)
        out[b, :, zh * ZS : (zh + 1) * ZS] = results[c]["y"].astype(np.float32)
    return out


def kernel(x, weight):
    from concourse.bass_utils import run_bass_kernel_spmd

    in_maps = _make_in_maps(x, weight)
    nc = _build_program()
    res = run_bass_kernel_spmd(nc, in_maps, list(range(N_CORES)))
    return _gather(res.results)


def _emulate_core(m):
    """Numpy model of one core incl. bf16 rounding of the AT chain."""
    X = np.asarray(m["xd"], np.float32)
    XE = np.asarray(m["xe"], np.float32)
    WK = np.asarray(m["wk"], np.float32)
    bf = lambda a: a.astype(ml_dtypes.bfloat16).astype(np.float32)
    y = np.zeros((128, ZS, DHW, DHW), np.float32)
    for zi in ZTILES:
        ps = np.zeros((NK, 128, NT), np.float32)
        for kk in range(NK):
            for p, (dz, dx, lo, hi, kind) in enumerate(KPASSES):
                j = kk * NP_K + p
                XX = XE if kind == "zdual" else X
                r = XX[lo:hi, kk, zi + dz : zi + dz + ZT, 0:NW, dx : dx + 24]
                ps[kk] += WK[lo:hi, j].T @ r.reshape(hi - lo, -1)
        m0 = bf(ps[0])
        m5 = bf(ps[5])
        a = bf(ps[1] + ps[2])
        b_ = bf(ps[1] - ps[2])
        pp = bf(ps[3] + ps[4])
        q = bf(ps[3] - ps[4])
        u = bf(a + pp)
        rows = [bf(u + m0), bf(2 * q + b_), bf(4 * pp + a), bf(bf(8 * q + b_) + m5)]
        yi = np.stack([r.reshape(128, ZT, NW, 24) for r in rows], axis=3)
        y[:, zi : zi + ZT] = yi.reshape(128, ZT, 24, 24)
    return y


if __name__ == "__main__":
    import jax

    sys.path.insert(0, "/root/problem")
    import reference

    cpu = jax.devices("cpu")[0]
    with jax.default_device(cpu):
        inputs = {k: np.asarray(v) for k, v in reference.setup_inputs().items()}
        expected = np.asarray(
            reference.reference(**{k: jax.device_put(v, cpu) for k, v in inputs.items()})
        )
    in_maps = _make_in_maps(inputs["x"], inputs["weight"])
    y = _emulate_core(in_maps[0])
    exp = expected[0][:, 0:ZS]
    err = np.linalg.norm(y - exp) / np.linalg.norm(exp)
    print("emulated core0 rel err:", err)


# revision 50
# speedup vs baseline: 1.6686x; 1.0317x over previous
"""Winograd F(4,3)-y conv3d. Measured ~32.1us/conv per core (baseline
56.7us on the same min-slope harness; quoted 68.9us previously).

The final 39.6->32.1us jump: the evacuation-tile pool had bufs=1 (an
SBUF-pressure cut made when an extra input tensor was resident), which
created a cross-engine WAR -- body k+1's ACT copy of m1/m3 had to wait
for body k's DVE reads, delaying PSUM bank releases on the PE critical
path. Dropping the xe tensor (measured exactly performance-neutral:
its 3.35MB DMA trades 1:1 against 24 extra matmuls) freed 51KB so the
evacuation tiles could double-buffer.

Per-core work: 4 z-tiles x 3 planes, N=432 per matmul (one PSUM bank),
6 winograd m-terms (kk) x 6 passes covering the 9 (dz,dx) taps as
3 x-duals + 3 singles, all from one xd=[T; T(+1x)] tensor -- 144
matmuls/conv. HW ablations: DMA alone ~15us/conv (5.2MB at the HBM
limit), compute pipeline ~30us (144 MMs at ~195-210ns effective incl.
issue/sem overhead); input DMA almost fully hides behind compute.

What mattered on real HW (sim cost model was misleading):
- For_i carries an all-engine barrier + sem reset per iteration, so
  bodies are unrolled 8x inside the loop (`inner`) and weights load
  once per block; per-conv time is the marginal slope.
- HWDGE DMAs execute FIFO per issuing ring: y-store rides the idle
  SWDGE/Pool ring and xe the ACT ring so nothing head-of-line blocks
  the next body's xd chunks on the SP ring. On-chip SBUF->SBUF shift
  copies (tried first) were far slower than shipping both host-shifted
  tensors.
- Pass-major matmul order across the 4 z-tiles: consecutive MMs share
  stationary weights, eliding 3/4 LDWEIGHTS (same-row-group LDW can't
  hide behind a running matmul).
- Inverse transform fused into evacuation: ACT copies m0/m1/m3/m5 out
  of PSUM, DVE forms a/b/p/q with one PSUM operand each (HW limit:
  max one PSUM input per DVE op), scalar_tensor_tensor fuses the
  *2/*4/*8 scaled adds, intermediates bf16 (2x DVE rate). All of this
  overlaps the PE stream completely (measured: removing it changes
  nothing).
"""

import sys

if "/opt/trn_rl_repo" not in sys.path:
    sys.path.insert(0, "/opt/trn_rl_repo")

import ml_dtypes
import numpy as np

CIN, COUT, K = 64, 128, 3
DHW = 24
ZS = 12  # z planes per core
NPL = 14  # input planes incl halo
PW = 26
NW = 6  # y window count (stride 4, size 6)
NK = 6  # winograd m-terms per window
N_CORES = 8
ZT = 3  # z planes per tile
ZTILES = (0, 3, 6, 9)
NT = ZT * NW * 24  # 432 cols per matmul

BT = np.array(
    [
        [4, 0, -5, 0, 1, 0],
        [0, -4, -4, 1, 1, 0],
        [0, 4, -4, -1, 1, 0],
        [0, -2, -1, 2, 1, 0],
        [0, 2, -1, -2, 1, 0],
        [0, 4, 0, -5, 0, 1],
    ],
    np.float32,
)
G = np.array(
    [
        [1 / 4, 0, 0],
        [-1 / 6, -1 / 6, -1 / 6],
        [-1 / 6, 1 / 6, -1 / 6],
        [1 / 24, 1 / 12, 1 / 6],
        [1 / 24, -1 / 12, 1 / 6],
        [0, 0, 1],
    ],
    np.float32,
)

# per-(kk,tile) passes: (dz, dx_ap, lo, hi). All passes read xd=[T; T(+1x)]:
# a [0:128] pass computes taps (dz, dx_ap) + (dz, dx_ap+1); a [0:64] pass
# the single lower tap (dz, dx_ap); a [64:128] pass the single upper tap
# (dz, dx_ap+1) via tile_position (64,0).
KPASSES = (
    (0, 0, 0, 128, "dual"),  # (0,0)+(0,1)
    (1, 0, 0, 128, "dual"),  # (1,0)+(1,1)
    (2, 0, 0, 128, "dual"),  # (2,0)+(2,1)
    (0, 2, 0, 64, "lo"),  # single (0,2)
    (1, 1, 0, 128, "up"),  # single (1,2): upper half only, zero lower weights
    (2, 2, 0, 64, "lo"),  # single (2,2)
)
NP_K = len(KPASSES)  # 6


def _elide_redundant_ldweights(nc):
    n_drop = 0
    for f in nc.m.functions:
        for b in f.blocks:
            last_key = None
            drop = []
            for inst in b.instructions:
                if type(inst).__name__ == "InstLdweights":
                    key = (str(inst.ins[0]), str(inst.perf_mode), str(inst.is_transpose))
                    si = inst.sync_info
                    clean = si is None or (len(si.on_wait) == 0 and len(si.on_update) == 0)
                    if key == last_key and clean:
                        drop.append(inst)
                    else:
                        last_key = key
            for inst in drop:
                b.instructions.remove(inst)
            n_drop += len(drop)
    return n_drop


def _build_program(loop_n=None, unroll=False, inner=1, bodies=1):
    import concourse.tile as tile
    from concourse import bacc, mybir

    BF16 = mybir.dt.bfloat16
    F32 = mybir.dt.float32
    MULT = mybir.AluOpType.mult
    ADD = mybir.AluOpType.add

    nc = bacc.Bacc("TRN2")
    xd_in = nc.declare_dram_parameter("xd", [128, NK, NPL, NW, PW], BF16, isOutput=False)
    wk_in = nc.declare_dram_parameter("wk", [128, NK * NP_K, 128], BF16, isOutput=False)
    y_out = nc.declare_dram_parameter("y", [128, ZS, DHW, DHW], BF16, isOutput=True)

    with tile.TileContext(nc) as tc:
        with (
            tc.tile_pool(name="xw", bufs=1) as xw_pool,
            tc.tile_pool(name="ps", bufs=8, space="PSUM") as ps_pool,
            tc.tile_pool(name="ev", bufs=2) as ev_pool,
            tc.tile_pool(name="ob", bufs=2) as ob_pool,
        ):

            def body(W):
                # 3 chunks of 2 kk each: few dma_starts (fixed cost ~2us each
                # on the ring) but the first matmuls still start early
                xdk = []
                for ck in range(3):
                    xd = xw_pool.tile(
                        [128, 2, NPL, NW, PW], BF16, name=f"xd{ck}", tag=f"xd{ck}"
                    )
                    nc.sync.dma_start(out=xd[:], in_=xd_in[:, 2 * ck : 2 * ck + 2])
                    xdk.append(xd)

                # psum accumulators, one bank per (tile, kk) group, evacuated
                # progressively so 8 banks suffice in kk-major order
                psq = {}
                evt = {}
                for kk in range(NK):
                    for t in range(4):
                        psq[(kk, t)] = ps_pool.tile(
                            [128, 512], F32, name="ps", tag="ps"
                        )
                    # pass-major over the 4 z-tiles: consecutive matmuls share
                    # the same stationary weights, so 3 of every 4 LDWEIGHTS
                    # get elided (same-row-group LDW cannot hide behind a
                    # running matmul, so each unelided one costs ~100ns)
                    for p, (dz, dx, lo, hi, kind) in enumerate(KPASSES):
                        j = kk * NP_K + p
                        for t, zi in enumerate(ZTILES):
                            rhs = xdk[kk // 2][
                                lo:hi, kk % 2, zi + dz : zi + dz + ZT, 0:NW,
                                dx : dx + 24,
                            ]
                            nc.tensor.matmul(
                                psq[(kk, t)][:, :NT],
                                lhsT=W[lo:hi, j, :],
                                rhs=rhs,
                                start=(p == 0),
                                stop=(p == NP_K - 1),
                                skip_group_check=True,
                            )
                    # progressive evacuation: frees the two source banks per op
                    for t in range(4):
                        ps = psq[(kk, t)]

                        def ev(nm, _t=t, dt=BF16):
                            tl = ev_pool.tile(
                                [128, NT], dt, name=f"{nm}{_t}", tag=f"{nm}{_t}"
                            )
                            evt[(nm, _t)] = tl
                            return tl

                        # DVE may read only ONE PSUM operand per op, so odd
                        # m-terms go through an ACT f32 copy first
                        if kk == 0:
                            m0 = ev("m0")
                            nc.scalar.copy(m0[:], ps[:, :NT])
                        elif kk == 1:
                            m1 = ev("m1", dt=F32)
                            nc.scalar.copy(m1[:], ps[:, :NT])
                        elif kk == 2:
                            a = ev("a")
                            nc.vector.tensor_add(
                                a[:], evt[("m1", t)][:], ps[:, :NT]
                            )
                            b = ev("b")
                            nc.vector.tensor_sub(
                                b[:], evt[("m1", t)][:], ps[:, :NT]
                            )
                        elif kk == 3:
                            m3 = ev("m3", dt=F32)
                            nc.scalar.copy(m3[:], ps[:, :NT])
                        elif kk == 4:
                            pp = ev("p")
                            nc.vector.tensor_add(
                                pp[:], evt[("m3", t)][:], ps[:, :NT]
                            )
                            q = ev("q")
                            nc.vector.tensor_sub(
                                q[:], evt[("m3", t)][:], ps[:, :NT]
                            )
                        elif kk == 5:
                            m5 = ev("m5")
                            nc.scalar.copy(m5[:], ps[:, :NT])

                # final combine per tile into one merged output buffer,
                # then a single y store (one dma_start instead of four)
                ob = ob_pool.tile([128, ZS, NW, 4, 24], BF16, name="ob", tag="ob")
                for t, zi in enumerate(ZTILES):
                    m0, m5, a, b, pp, q = (
                        evt[(nm, t)] for nm in ("m0", "m5", "a", "b", "p", "q")
                    )
                    u = ev_pool.tile([128, NT], BF16, name=f"u{t}", tag=f"u{t}")
                    nc.vector.tensor_add(u[:], a[:], pp[:])
                    nc.vector.tensor_add(ob[:, zi : zi + ZT, :, 0, :], u[:], m0[:])
                    nc.vector.scalar_tensor_tensor(
                        ob[:, zi : zi + ZT, :, 1, :], q[:], 2.0, b[:], MULT, ADD
                    )
                    nc.vector.scalar_tensor_tensor(
                        ob[:, zi : zi + ZT, :, 2, :], pp[:], 4.0, a[:], MULT, ADD
                    )
                    t2 = ev_pool.tile([128, NT], BF16, name=f"t{t}", tag=f"t{t}")
                    nc.vector.scalar_tensor_tensor(t2[:], q[:], 8.0, b[:], MULT, ADD)
                    nc.vector.tensor_add(ob[:, zi : zi + ZT, :, 3, :], t2[:], m5[:])
                # y store on the idle SWDGE/Pool ring so it never head-of-line
                # blocks the next body's xd loads (SP) or W/copies (ACT)
                nc.gpsimd.dma_start(out=y_out[:], in_=ob[:])

            def block(n_bodies):
                # weights are loop-invariant: one load per block
                W = xw_pool.tile([128, NK * NP_K, 128], BF16, name="W", tag="W")
                nc.scalar.dma_start(out=W[:], in_=wk_in[:])
                for _ in range(n_bodies):
                    body(W)

            if loop_n is not None:
                if unroll:
                    for _k in range(loop_n):
                        block(1)
                else:
                    with tc.For_i(0, loop_n, 1) as _i:
                        block(inner)
            else:
                block(bodies)

    nc.finalize()
    _elide_redundant_ldweights(nc)
    return nc


def _wtap(gw, kk, dz, dx):
    return gw[kk, :, :, dz, dx].T


def _transform_w(weight):
    w = np.asarray(weight, np.float32).reshape(COUT, CIN, K, K, K)
    gw = np.einsum("ky,oczyx->koczx", G, w)  # (6, O, C, 3z, 3x)
    wk = np.zeros((128, NK * NP_K, 128), np.float32)
    for kk in range(NK):
        for p, (dz, dx, lo, hi, kind) in enumerate(KPASSES):
            j = kk * NP_K + p
            if kind == "dual":  # (dz, dx) + (dz, dx+1)
                wk[0:64, j] = _wtap(gw, kk, dz, dx)
                wk[64:128, j] = _wtap(gw, kk, dz, dx + 1)
            elif kind == "lo":  # lower single (dz, dx)
                wk[0:64, j] = _wtap(gw, kk, dz, dx)
            else:  # upper single (dz, dx+1); lower rows stay zero
                wk[64:128, j] = _wtap(gw, kk, dz, dx + 1)
    return wk.astype(ml_dtypes.bfloat16)


def _make_in_maps(x, weight):
    wk = _transform_w(weight)
    x = np.asarray(x, np.float32)
    in_maps = []
    for c in range(N_CORES):
        b, zh = divmod(c, 2)
        z0 = zh * ZS
        xpad = np.zeros((CIN, PW, PW, PW), np.float32)
        xpad[:, 1:25, 1:25, 1:25] = x[b]
        win = xpad[:, z0 : z0 + NPL]  # (64, 14, 26, 26)
        # T[c, k, z, w, x] = sum_j BT[k, j] win[c, z, 4w+j, x]
        wmat = np.lib.stride_tricks.sliding_window_view(win, 6, axis=2)[:, :, ::4][
            :, :, :NW
        ]
        T = np.einsum("kj,czwxj->ckzwx", BT, wmat)  # (64, 6, 14, 6, 26)
        X = np.zeros((128, NK, NPL, NW, PW), np.float32)
        X[0:64] = T
        X[64:128, :, :, :, :-1] = T[:, :, :, :, 1:]  # +1x shift
        in_maps.append({"wk": wk, "xd": X.astype(ml_dtypes.bfloat16)})
    return in_maps


def _gather(results):
    out = np.empty((4, COUT, DHW, DHW, DHW), np.float32)
    for c in range(N_CORES):
        b, zh = divmod(c, 2)
        out[b, :, zh * ZS : (zh + 1) * ZS] = results[c]["y"].astype(np.float32)
    return out


def kernel(x, weight):
    from concourse.bass_utils import run_bass_kernel_spmd

    in_maps = _make_in_maps(x, weight)
    nc = _build_program()
    res = run_bass_kernel_spmd(nc, in_maps, list(range(N_CORES)))
    return _gather(res.results)


def _emulate_core(m):
    """Numpy model of one core incl. bf16 rounding of the AT chain."""
    X = np.asarray(m["xd"], np.float32)
    WK = np.asarray(m["wk"], np.float32)
    bf = lambda a: a.astype(ml_dtypes.bfloat16).astype(np.float32)
    y = np.zeros((128, ZS, DHW, DHW), np.float32)
    for zi in ZTILES:
        ps = np.zeros((NK, 128, NT), np.float32)
        for kk in range(NK):
            for p, (dz, dx, lo, hi, kind) in enumerate(KPASSES):
                j = kk * NP_K + p
                r = X[lo:hi, kk, zi + dz : zi + dz + ZT, 0:NW, dx : dx + 24]
                ps[kk] += WK[lo:hi, j].T @ r.reshape(hi - lo, -1)
        m0 = bf(ps[0])
        m5 = bf(ps[5])
        a = bf(ps[1] + ps[2])
        b_ = bf(ps[1] - ps[2])
        pp = bf(ps[3] + ps[4])
        q = bf(ps[3] - ps[4])
        u = bf(a + pp)
        rows = [bf(u + m0), bf(2 * q + b_), bf(4 * pp + a), bf(bf(8 * q + b_) + m5)]
        yi = np.stack([r.reshape(128, ZT, NW, 24) for r in rows], axis=3)
        y[:, zi : zi + ZT] = yi.reshape(128, ZT, 24, 24)
    return y


if __name__ == "__main__":
    import jax

    sys.path.insert(0, "/root/problem")
    import reference

    cpu = jax.devices("cpu")[0]
    with jax.default_device(cpu):
        inputs = {k: np.asarray(v) for k, v in reference.setup_inputs().items()}
        expected = np.asarray(
            reference.reference(**{k: jax.device_put(v, cpu) for k, v in inputs.items()})
        )
    in_maps = _make_in_maps(inputs["x"], inputs["weight"])
    y = _emulate_core(in_maps[0])
    exp = expected[0][:, 0:ZS]
    err = np.linalg.norm(y - exp) / np.linalg.norm(exp)
    print("emulated core0 rel err:", err)


# revision 52
# speedup vs baseline: 1.7537x; 1.0510x over previous
"""Winograd F(4,3)-y conv3d. Measured ~32.1us/conv per core (baseline
56.7us on the same min-slope harness; quoted 68.9us previously).

The final 39.6->32.1us jump: the evacuation-tile pool had bufs=1 (an
SBUF-pressure cut made when an extra input tensor was resident), which
created a cross-engine WAR -- body k+1's ACT copy of m1/m3 had to wait
for body k's DVE reads, delaying PSUM bank releases on the PE critical
path. Dropping the xe tensor (measured exactly performance-neutral:
its 3.35MB DMA trades 1:1 against 24 extra matmuls) freed 51KB so the
evacuation tiles could double-buffer.

Per-core work: 4 z-tiles x 3 planes, N=432 per matmul (one PSUM bank),
6 winograd m-terms (kk) x 6 passes covering the 9 (dz,dx) taps as
3 x-duals + 3 singles, all from one xd=[T; T(+1x)] tensor -- 144
matmuls/conv. HW ablations: DMA alone ~15us/conv (5.2MB at the HBM
limit), compute pipeline ~30us (144 MMs at ~195-210ns effective incl.
issue/sem overhead); input DMA almost fully hides behind compute.

What mattered on real HW (sim cost model was misleading):
- For_i carries an all-engine barrier + sem reset per iteration, so
  bodies are unrolled 8x inside the loop (`inner`) and weights load
  once per block; per-conv time is the marginal slope.
- HWDGE DMAs execute FIFO per issuing ring: y-store rides the idle
  SWDGE/Pool ring and xe the ACT ring so nothing head-of-line blocks
  the next body's xd chunks on the SP ring. On-chip SBUF->SBUF shift
  copies (tried first) were far slower than shipping both host-shifted
  tensors.
- Pass-major matmul order across the 4 z-tiles: consecutive MMs share
  stationary weights, eliding 3/4 LDWEIGHTS (same-row-group LDW can't
  hide behind a running matmul).
- Inverse transform fused into evacuation: ACT copies m0/m1/m3/m5 out
  of PSUM, DVE forms a/b/p/q with one PSUM operand each (HW limit:
  max one PSUM input per DVE op), scalar_tensor_tensor fuses the
  *2/*4/*8 scaled adds, intermediates bf16 (2x DVE rate). All of this
  overlaps the PE stream completely (measured: removing it changes
  nothing).
"""

import sys

if "/opt/trn_rl_repo" not in sys.path:
    sys.path.insert(0, "/opt/trn_rl_repo")

import ml_dtypes
import numpy as np

CIN, COUT, K = 64, 128, 3
DHW = 24
ZS = 12  # z planes per core
NPL = 14  # input planes incl halo
PW = 26
NW = 6  # y window count (stride 4, size 6)
NK = 6  # winograd m-terms per window
N_CORES = 8
ZT = 3  # z planes per tile
ZTILES = (0, 3, 6, 9)
NT = ZT * NW * 24  # 432 cols per matmul

BT = np.array(
    [
        [4, 0, -5, 0, 1, 0],
        [0, -4, -4, 1, 1, 0],
        [0, 4, -4, -1, 1, 0],
        [0, -2, -1, 2, 1, 0],
        [0, 2, -1, -2, 1, 0],
        [0, 4, 0, -5, 0, 1],
    ],
    np.float32,
)
G = np.array(
    [
        [1 / 4, 0, 0],
        [-1 / 6, -1 / 6, -1 / 6],
        [-1 / 6, 1 / 6, -1 / 6],
        [1 / 24, 1 / 12, 1 / 6],
        [1 / 24, -1 / 12, 1 / 6],
        [0, 0, 1],
    ],
    np.float32,
)

# per-(kk,tile) passes: (dz, dx_ap, lo, hi). All passes read xd=[T; T(+1x)]:
# a [0:128] pass computes taps (dz, dx_ap) + (dz, dx_ap+1); a [0:64] pass
# the single lower tap (dz, dx_ap); a [64:128] pass the single upper tap
# (dz, dx_ap+1) via tile_position (64,0).
KPASSES = (
    (0, 0, 0, 128, "dual"),  # (0,0)+(0,1)
    (1, 0, 0, 128, "dual"),  # (1,0)+(1,1)
    (2, 0, 0, 128, "dual"),  # (2,0)+(2,1)
    (0, 2, 0, 64, "lo"),  # single (0,2)
    (1, 1, 0, 128, "up"),  # single (1,2): upper half only, zero lower weights
    (2, 2, 0, 64, "lo"),  # single (2,2)
)
NP_K = len(KPASSES)  # 6


def _elide_redundant_ldweights(nc):
    n_drop = 0
    for f in nc.m.functions:
        for b in f.blocks:
            last_key = None
            drop = []
            for inst in b.instructions:
                if type(inst).__name__ == "InstLdweights":
                    key = (str(inst.ins[0]), str(inst.perf_mode), str(inst.is_transpose))
                    si = inst.sync_info
                    clean = si is None or (len(si.on_wait) == 0 and len(si.on_update) == 0)
                    if key == last_key and clean:
                        drop.append(inst)
                    else:
                        last_key = key
            for inst in drop:
                b.instructions.remove(inst)
            n_drop += len(drop)
    return n_drop


def _build_program(loop_n=None, unroll=False, inner=1, bodies=1):
    import concourse.tile as tile
    from concourse import bacc, mybir

    BF16 = mybir.dt.bfloat16
    F32 = mybir.dt.float32
    MULT = mybir.AluOpType.mult
    ADD = mybir.AluOpType.add

    nc = bacc.Bacc("TRN2")
    xd_in = nc.declare_dram_parameter("xd", [128, NK, NPL, NW, PW], BF16, isOutput=False)
    wk_in = nc.declare_dram_parameter("wk", [128, NK * NP_K, 128], BF16, isOutput=False)
    y_out = nc.declare_dram_parameter("y", [128, ZS, DHW, DHW], BF16, isOutput=True)

    with tile.TileContext(nc) as tc:
        with (
            tc.tile_pool(name="xw", bufs=1) as xw_pool,
            tc.tile_pool(name="ps", bufs=8, space="PSUM") as ps_pool,
            tc.tile_pool(name="ev", bufs=2) as ev_pool,
            tc.tile_pool(name="ob", bufs=2) as ob_pool,
        ):

            def body(W):
                # 3 chunks of 2 kk each: few dma_starts (fixed cost ~2us each
                # on the ring) but the first matmuls still start early
                xdk = []
                for ck in range(3):
                    xd = xw_pool.tile(
                        [128, 2, NPL, NW, PW], BF16, name=f"xd{ck}", tag=f"xd{ck}"
                    )
                    nc.sync.dma_start(out=xd[:], in_=xd_in[:, 2 * ck : 2 * ck + 2])
                    xdk.append(xd)

                # psum accumulators, one bank per (tile, kk) group, evacuated
                # progressively so 8 banks suffice in kk-major order
                psq = {}
                evt = {}
                for kk in range(NK):
                    for t in range(4):
                        psq[(kk, t)] = ps_pool.tile(
                            [128, 512], F32, name="ps", tag="ps"
                        )
                    # pass-major over the 4 z-tiles: consecutive matmuls share
                    # the same stationary weights, so 3 of every 4 LDWEIGHTS
                    # get elided (same-row-group LDW cannot hide behind a
                    # running matmul, so each unelided one costs ~100ns)
                    for p, (dz, dx, lo, hi, kind) in enumerate(KPASSES):
                        j = kk * NP_K + p
                        for t, zi in enumerate(ZTILES):
                            rhs = xdk[kk // 2][
                                lo:hi, kk % 2, zi + dz : zi + dz + ZT, 0:NW,
                                dx : dx + 24,
                            ]
                            nc.tensor.matmul(
                                psq[(kk, t)][:, :NT],
                                lhsT=W[lo:hi, j, :],
                                rhs=rhs,
                                start=(p == 0),
                                stop=(p == NP_K - 1),
                                skip_group_check=True,
                            )
                    # progressive evacuation: frees the two source banks per op
                    for t in range(4):
                        ps = psq[(kk, t)]

                        def ev(nm, _t=t, dt=BF16):
                            tl = ev_pool.tile(
                                [128, NT], dt, name=f"{nm}{_t}", tag=f"{nm}{_t}"
                            )
                            evt[(nm, _t)] = tl
                            return tl

                        # ACT copies every m-term out of PSUM (bank release
                        # rides the fast ACT FIFO, decoupled from DVE's
                        # queue); DVE then pairs them at 2x bf16 rate
                        mk = ev(f"m{kk}")
                        nc.scalar.copy(mk[:], ps[:, :NT])
                        if kk == 2:
                            a = ev("a")
                            nc.vector.tensor_add(
                                a[:], evt[("m1", t)][:], evt[("m2", t)][:]
                            )
                            b = ev("b")
                            nc.vector.tensor_sub(
                                b[:], evt[("m1", t)][:], evt[("m2", t)][:]
                            )
                        elif kk == 4:
                            pp = ev("p")
                            nc.vector.tensor_add(
                                pp[:], evt[("m3", t)][:], evt[("m4", t)][:]
                            )
                            q = ev("q")
                            nc.vector.tensor_sub(
                                q[:], evt[("m3", t)][:], evt[("m4", t)][:]
                            )

                # final combine per tile into one merged output buffer,
                # then a single y store (one dma_start instead of four)
                ob = ob_pool.tile([128, ZS, NW, 4, 24], BF16, name="ob", tag="ob")
                for t, zi in enumerate(ZTILES):
                    m0, m5, a, b, pp, q = (
                        evt[(nm, t)] for nm in ("m0", "m5", "a", "b", "p", "q")
                    )
                    u = ev_pool.tile([128, NT], BF16, name=f"u{t}", tag=f"u{t}")
                    nc.vector.tensor_add(u[:], a[:], pp[:])
                    nc.vector.tensor_add(ob[:, zi : zi + ZT, :, 0, :], u[:], m0[:])
                    nc.vector.scalar_tensor_tensor(
                        ob[:, zi : zi + ZT, :, 1, :], q[:], 2.0, b[:], MULT, ADD
                    )
                    nc.vector.scalar_tensor_tensor(
                        ob[:, zi : zi + ZT, :, 2, :], pp[:], 4.0, a[:], MULT, ADD
                    )
                    t2 = ev_pool.tile([128, NT], BF16, name=f"t{t}", tag=f"t{t}")
                    nc.vector.scalar_tensor_tensor(t2[:], q[:], 8.0, b[:], MULT, ADD)
                    nc.vector.tensor_add(ob[:, zi : zi + ZT, :, 3, :], t2[:], m5[:])
                # y store on the idle SWDGE/Pool ring so it never head-of-line
                # blocks the next body's xd loads (SP) or W/copies (ACT)
                nc.gpsimd.dma_start(out=y_out[:], in_=ob[:])

            def block(n_bodies):
                # weights are loop-invariant: one load per block
                W = xw_pool.tile([128, NK * NP_K, 128], BF16, name="W", tag="W")
                nc.scalar.dma_start(out=W[:], in_=wk_in[:])
                for _ in range(n_bodies):
                    body(W)

            if loop_n is not None:
                if unroll:
                    for _k in range(loop_n):
                        block(1)
                else:
                    with tc.For_i(0, loop_n, 1) as _i:
                        block(inner)
            else:
                block(bodies)

    nc.finalize()
    _elide_redundant_ldweights(nc)
    return nc


def _wtap(gw, kk, dz, dx):
    return gw[kk, :, :, dz, dx].T


def _transform_w(weight):
    w = np.asarray(weight, np.float32).reshape(COUT, CIN, K, K, K)
    gw = np.einsum("ky,oczyx->koczx", G, w)  # (6, O, C, 3z, 3x)
    wk = np.zeros((128, NK * NP_K, 128), np.float32)
    for kk in range(NK):
        for p, (dz, dx, lo, hi, kind) in enumerate(KPASSES):
            j = kk * NP_K + p
            if kind == "dual":  # (dz, dx) + (dz, dx+1)
                wk[0:64, j] = _wtap(gw, kk, dz, dx)
                wk[64:128, j] = _wtap(gw, kk, dz, dx + 1)
            elif kind == "lo":  # lower single (dz, dx)
                wk[0:64, j] = _wtap(gw, kk, dz, dx)
            else:  # upper single (dz, dx+1); lower rows stay zero
                wk[64:128, j] = _wtap(gw, kk, dz, dx + 1)
    return wk.astype(ml_dtypes.bfloat16)


def _make_in_maps(x, weight):
    wk = _transform_w(weight)
    x = np.asarray(x, np.float32)
    in_maps = []
    for c in range(N_CORES):
        b, zh = divmod(c, 2)
        z0 = zh * ZS
        xpad = np.zeros((CIN, PW, PW, PW), np.float32)
        xpad[:, 1:25, 1:25, 1:25] = x[b]
        win = xpad[:, z0 : z0 + NPL]  # (64, 14, 26, 26)
        # T[c, k, z, w, x] = sum_j BT[k, j] win[c, z, 4w+j, x]
        wmat = np.lib.stride_tricks.sliding_window_view(win, 6, axis=2)[:, :, ::4][
            :, :, :NW
        ]
        T = np.einsum("kj,czwxj->ckzwx", BT, wmat)  # (64, 6, 14, 6, 26)
        X = np.zeros((128, NK, NPL, NW, PW), np.float32)
        X[0:64] = T
        X[64:128, :, :, :, :-1] = T[:, :, :, :, 1:]  # +1x shift
        in_maps.append({"wk": wk, "xd": X.astype(ml_dtypes.bfloat16)})
    return in_maps


def _gather(results):
    out = np.empty((4, COUT, DHW, DHW, DHW), np.float32)
    for c in range(N_CORES):
        b, zh = divmod(c, 2)
        out[b, :, zh * ZS : (zh + 1) * ZS] = results[c]["y"].astype(np.float32)
    return out


def kernel(x, weight):
    from concourse.bass_utils import run_bass_kernel_spmd

    in_maps = _make_in_maps(x, weight)
    nc = _build_program()
    res = run_bass_kernel_spmd(nc, in_maps, list(range(N_CORES)))
    return _gather(res.results)


def _emulate_core(m):
    """Numpy model of one core incl. bf16 rounding of the AT chain."""
    X = np.asarray(m["xd"], np.float32)
    WK = np.asarray(m["wk"], np.float32)
    bf = lambda a: a.astype(ml_dtypes.bfloat16).astype(np.float32)
    y = np.zeros((128, ZS, DHW, DHW), np.float32)
    for zi in ZTILES:
        ps = np.zeros((NK, 128, NT), np.float32)
        for kk in range(NK):
            for p, (dz, dx, lo, hi, kind) in enumerate(KPASSES):
                j = kk * NP_K + p
                r = X[lo:hi, kk, zi + dz : zi + dz + ZT, 0:NW, dx : dx + 24]
                ps[kk] += WK[lo:hi, j].T @ r.reshape(hi - lo, -1)
        m0 = bf(ps[0])
        m5 = bf(ps[5])
        m1, m2, m3, m4 = bf(ps[1]), bf(ps[2]), bf(ps[3]), bf(ps[4])
        a = bf(m1 + m2)
        b_ = bf(m1 - m2)
        pp = bf(m3 + m4)
        q = bf(m3 - m4)
        u = bf(a + pp)
        rows = [bf(u + m0), bf(2 * q + b_), bf(4 * pp + a), bf(bf(8 * q + b_) + m5)]
        yi = np.stack([r.reshape(128, ZT, NW, 24) for r in rows], axis=3)
        y[:, zi : zi + ZT] = yi.reshape(128, ZT, 24, 24)
    return y


if __name__ == "__main__":
    import jax

    sys.path.insert(0, "/root/problem")
    import reference

    cpu = jax.devices("cpu")[0]
    with jax.default_device(cpu):
        inputs = {k: np.asarray(v) for k, v in reference.setup_inputs().items()}
        expected = np.asarray(
            reference.reference(**{k: jax.device_put(v, cpu) for k, v in inputs.items()})
        )
    in_maps = _make_in_maps(inputs["x"], inputs["weight"])
    y = _emulate_core(in_maps[0])
    exp = expected[0][:, 0:ZS]
    err = np.linalg.norm(y - exp) / np.linalg.norm(exp)
    print("emulated core0 rel err:", err)


# revision 54
# speedup vs baseline: 1.8140x; 1.0344x over previous
"""Winograd F(4,3)-y conv3d. Measured ~36.6-37.1us/conv per core
(baseline 56.7us on the same min-slope harness; 68.9us as originally
quoted).

The 39.6->37us step came from two changes: (1) the evacuation-tile
pool had bufs=1 (an SBUF-pressure cut made when an extra input tensor
was resident), creating a cross-engine WAR that delayed PSUM bank
releases on the PE critical path -- dropping the xe tensor (measured
performance-neutral: its 3.35MB DMA trades 1:1 against 24 extra
matmuls) freed 51KB so evacuation tiles double-buffer; (2) ACT copies
ALL six m-terms out of PSUM (bank release rides the fast ACT FIFO)
and DVE pairs them at 2x bf16 rate.

Per-core work: 4 z-tiles x 3 planes, N=432 per matmul (one PSUM bank),
6 winograd m-terms (kk) x 6 passes covering the 9 (dz,dx) taps as
3 x-duals + 3 singles, all from one xd=[T; T(+1x)] tensor -- 144
matmuls/conv. HW ablations: DMA alone ~15us/conv (5.2MB at the HBM
limit), compute pipeline ~30us (144 MMs at ~195-210ns effective incl.
issue/sem overhead); input DMA almost fully hides behind compute.

What mattered on real HW (sim cost model was misleading):
- For_i carries an all-engine barrier + sem reset per iteration, so
  bodies are unrolled 8x inside the loop (`inner`) and weights load
  once per block; per-conv time is the marginal slope.
- HWDGE DMAs execute FIFO per issuing ring: y-store rides the idle
  SWDGE/Pool ring and xe the ACT ring so nothing head-of-line blocks
  the next body's xd chunks on the SP ring. On-chip SBUF->SBUF shift
  copies (tried first) were far slower than shipping both host-shifted
  tensors.
- Pass-major matmul order across the 4 z-tiles: consecutive MMs share
  stationary weights, eliding 3/4 LDWEIGHTS (same-row-group LDW can't
  hide behind a running matmul).
- Inverse transform fused into evacuation: ACT copies m0/m1/m3/m5 out
  of PSUM, DVE forms a/b/p/q with one PSUM operand each (HW limit:
  max one PSUM input per DVE op), scalar_tensor_tensor fuses the
  *2/*4/*8 scaled adds, intermediates bf16 (2x DVE rate). All of this
  overlaps the PE stream completely (measured: removing it changes
  nothing).
"""

import sys

if "/opt/trn_rl_repo" not in sys.path:
    sys.path.insert(0, "/opt/trn_rl_repo")

import ml_dtypes
import numpy as np

CIN, COUT, K = 64, 128, 3
DHW = 24
ZS = 12  # z planes per core
NPL = 14  # input planes incl halo
PW = 26
NW = 6  # y window count (stride 4, size 6)
NK = 6  # winograd m-terms per window
N_CORES = 8
ZT = 3  # z planes per tile
ZTILES = (0, 3, 6, 9)
NT = ZT * NW * 24  # 432 cols per matmul

BT = np.array(
    [
        [4, 0, -5, 0, 1, 0],
        [0, -4, -4, 1, 1, 0],
        [0, 4, -4, -1, 1, 0],
        [0, -2, -1, 2, 1, 0],
        [0, 2, -1, -2, 1, 0],
        [0, 4, 0, -5, 0, 1],
    ],
    np.float32,
)
G = np.array(
    [
        [1 / 4, 0, 0],
        [-1 / 6, -1 / 6, -1 / 6],
        [-1 / 6, 1 / 6, -1 / 6],
        [1 / 24, 1 / 12, 1 / 6],
        [1 / 24, -1 / 12, 1 / 6],
        [0, 0, 1],
    ],
    np.float32,
)

# per-(kk,tile) passes: (dz, dx_ap, lo, hi). All passes read xd=[T; T(+1x)]:
# a [0:128] pass computes taps (dz, dx_ap) + (dz, dx_ap+1); a [0:64] pass
# the single lower tap (dz, dx_ap); a [64:128] pass the single upper tap
# (dz, dx_ap+1) via tile_position (64,0).
KPASSES = (
    (0, 0, 0, 128, "dual"),  # (0,0)+(0,1)
    (1, 0, 0, 128, "dual"),  # (1,0)+(1,1)
    (2, 0, 0, 128, "dual"),  # (2,0)+(2,1)
    (0, 2, 0, 128, "lo"),  # single (0,2): zero upper weights (keeps FWL on)
    (1, 1, 0, 128, "up"),  # single (1,2): upper half only, zero lower weights
    (2, 2, 0, 128, "lo"),  # single (2,2): zero upper weights
)
NP_K = len(KPASSES)  # 6


def _elide_redundant_ldweights(nc):
    n_drop = 0
    for f in nc.m.functions:
        for b in f.blocks:
            last_key = None
            drop = []
            for inst in b.instructions:
                if type(inst).__name__ == "InstLdweights":
                    key = (str(inst.ins[0]), str(inst.perf_mode), str(inst.is_transpose))
                    si = inst.sync_info
                    clean = si is None or (len(si.on_wait) == 0 and len(si.on_update) == 0)
                    if key == last_key and clean:
                        drop.append(inst)
                    else:
                        last_key = key
            for inst in drop:
                b.instructions.remove(inst)
            n_drop += len(drop)
    return n_drop


def _build_program(loop_n=None, unroll=False, inner=1, bodies=1):
    import concourse.tile as tile
    from concourse import bacc, mybir

    BF16 = mybir.dt.bfloat16
    F32 = mybir.dt.float32
    MULT = mybir.AluOpType.mult
    ADD = mybir.AluOpType.add

    nc = bacc.Bacc("TRN2")
    xd_in = nc.declare_dram_parameter("xd", [128, NK, NPL, NW, PW], BF16, isOutput=False)
    wk_in = nc.declare_dram_parameter("wk", [128, NK * NP_K, 128], BF16, isOutput=False)
    y_out = nc.declare_dram_parameter("y", [128, ZS, DHW, DHW], BF16, isOutput=True)

    with tile.TileContext(nc) as tc:
        with (
            tc.tile_pool(name="xw", bufs=1) as xw_pool,
            tc.tile_pool(name="ps", bufs=8, space="PSUM") as ps_pool,
            tc.tile_pool(name="ev", bufs=2) as ev_pool,
            tc.tile_pool(name="ob", bufs=2) as ob_pool,
        ):

            def body(W):
                # 3 chunks of 2 kk each: few dma_starts (fixed cost ~2us each
                # on the ring) but the first matmuls still start early
                xdk = []
                for ck in range(3):
                    xd = xw_pool.tile(
                        [128, 2, NPL, NW, PW], BF16, name=f"xd{ck}", tag=f"xd{ck}"
                    )
                    nc.sync.dma_start(out=xd[:], in_=xd_in[:, 2 * ck : 2 * ck + 2])
                    xdk.append(xd)

                # psum accumulators, one bank per (tile, kk) group, evacuated
                # progressively so 8 banks suffice in kk-major order
                psq = {}
                evt = {}
                for kk in range(NK):
                    for t in range(4):
                        psq[(kk, t)] = ps_pool.tile(
                            [128, 512], F32, name="ps", tag="ps"
                        )
                    # pass-major over the 4 z-tiles: consecutive matmuls share
                    # the same stationary weights, so 3 of every 4 LDWEIGHTS
                    # get elided (same-row-group LDW cannot hide behind a
                    # running matmul, so each unelided one costs ~100ns)
                    for p, (dz, dx, lo, hi, kind) in enumerate(KPASSES):
                        j = kk * NP_K + p
                        for t, zi in enumerate(ZTILES):
                            rhs = xdk[kk // 2][
                                lo:hi, kk % 2, zi + dz : zi + dz + ZT, 0:NW,
                                dx : dx + 24,
                            ]
                            nc.tensor.matmul(
                                psq[(kk, t)][:, :NT],
                                lhsT=W[lo:hi, j, :],
                                rhs=rhs,
                                start=(p == 0),
                                stop=(p == NP_K - 1),
                                skip_group_check=True,
                            )
                    # progressive evacuation: frees the two source banks per op
                    for t in range(4):
                        ps = psq[(kk, t)]

                        def ev(nm, _t=t, dt=BF16):
                            tl = ev_pool.tile(
                                [128, NT], dt, name=f"{nm}{_t}", tag=f"{nm}{_t}"
                            )
                            evt[(nm, _t)] = tl
                            return tl

                        # ACT copies every m-term out of PSUM (bank release
                        # rides the fast ACT FIFO, decoupled from DVE's
                        # queue); DVE then pairs them at 2x bf16 rate
                        mk = ev(f"m{kk}")
                        nc.scalar.copy(mk[:], ps[:, :NT])
                        if kk == 2:
                            a = ev("a")
                            nc.vector.tensor_add(
                                a[:], evt[("m1", t)][:], evt[("m2", t)][:]
                            )
                            b = ev("b")
                            nc.vector.tensor_sub(
                                b[:], evt[("m1", t)][:], evt[("m2", t)][:]
                            )
                        elif kk == 4:
                            pp = ev("p")
                            nc.vector.tensor_add(
                                pp[:], evt[("m3", t)][:], evt[("m4", t)][:]
                            )
                            q = ev("q")
                            nc.vector.tensor_sub(
                                q[:], evt[("m3", t)][:], evt[("m4", t)][:]
                            )

                # final combine per tile into one merged output buffer,
                # then a single y store (one dma_start instead of four)
                ob = ob_pool.tile([128, ZS, NW, 4, 24], BF16, name="ob", tag="ob")
                for t, zi in enumerate(ZTILES):
                    m0, m5, a, b, pp, q = (
                        evt[(nm, t)] for nm in ("m0", "m5", "a", "b", "p", "q")
                    )
                    u = ev_pool.tile([128, NT], BF16, name=f"u{t}", tag=f"u{t}")
                    nc.vector.tensor_add(u[:], a[:], pp[:])
                    nc.vector.tensor_add(ob[:, zi : zi + ZT, :, 0, :], u[:], m0[:])
                    nc.vector.scalar_tensor_tensor(
                        ob[:, zi : zi + ZT, :, 1, :], q[:], 2.0, b[:], MULT, ADD
                    )
                    nc.vector.scalar_tensor_tensor(
                        ob[:, zi : zi + ZT, :, 2, :], pp[:], 4.0, a[:], MULT, ADD
                    )
                    t2 = ev_pool.tile([128, NT], BF16, name=f"t{t}", tag=f"t{t}")
                    nc.vector.scalar_tensor_tensor(t2[:], q[:], 8.0, b[:], MULT, ADD)
                    nc.vector.tensor_add(ob[:, zi : zi + ZT, :, 3, :], t2[:], m5[:])
                # y store on the idle SWDGE/Pool ring so it never head-of-line
                # blocks the next body's xd loads (SP) or W/copies (ACT)
                nc.gpsimd.dma_start(out=y_out[:], in_=ob[:])

            def block(n_bodies):
                # weights are loop-invariant: one load per block
                W = xw_pool.tile([128, NK * NP_K, 128], BF16, name="W", tag="W")
                nc.scalar.dma_start(out=W[:], in_=wk_in[:])
                for _ in range(n_bodies):
                    body(W)

            if loop_n is not None:
                if unroll:
                    for _k in range(loop_n):
                        block(1)
                else:
                    with tc.For_i(0, loop_n, 1) as _i:
                        block(inner)
            else:
                block(bodies)

    nc.finalize()
    _elide_redundant_ldweights(nc)
    return nc


def _wtap(gw, kk, dz, dx):
    return gw[kk, :, :, dz, dx].T


def _transform_w(weight):
    w = np.asarray(weight, np.float32).reshape(COUT, CIN, K, K, K)
    gw = np.einsum("ky,oczyx->koczx", G, w)  # (6, O, C, 3z, 3x)
    wk = np.zeros((128, NK * NP_K, 128), np.float32)
    for kk in range(NK):
        for p, (dz, dx, lo, hi, kind) in enumerate(KPASSES):
            j = kk * NP_K + p
            if kind == "dual":  # (dz, dx) + (dz, dx+1)
                wk[0:64, j] = _wtap(gw, kk, dz, dx)
                wk[64:128, j] = _wtap(gw, kk, dz, dx + 1)
            elif kind == "lo":  # lower single (dz, dx)
                wk[0:64, j] = _wtap(gw, kk, dz, dx)
            else:  # upper single (dz, dx+1); lower rows stay zero
                wk[64:128, j] = _wtap(gw, kk, dz, dx + 1)
    return wk.astype(ml_dtypes.bfloat16)


def _make_in_maps(x, weight):
    wk = _transform_w(weight)
    x = np.asarray(x, np.float32)
    in_maps = []
    for c in range(N_CORES):
        b, zh = divmod(c, 2)
        z0 = zh * ZS
        xpad = np.zeros((CIN, PW, PW, PW), np.float32)
        xpad[:, 1:25, 1:25, 1:25] = x[b]
        win = xpad[:, z0 : z0 + NPL]  # (64, 14, 26, 26)
        # T[c, k, z, w, x] = sum_j BT[k, j] win[c, z, 4w+j, x]
        wmat = np.lib.stride_tricks.sliding_window_view(win, 6, axis=2)[:, :, ::4][
            :, :, :NW
        ]
        T = np.einsum("kj,czwxj->ckzwx", BT, wmat)  # (64, 6, 14, 6, 26)
        X = np.zeros((128, NK, NPL, NW, PW), np.float32)
        X[0:64] = T
        X[64:128, :, :, :, :-1] = T[:, :, :, :, 1:]  # +1x shift
        in_maps.append({"wk": wk, "xd": X.astype(ml_dtypes.bfloat16)})
    return in_maps


def _gather(results):
    out = np.empty((4, COUT, DHW, DHW, DHW), np.float32)
    for c in range(N_CORES):
        b, zh = divmod(c, 2)
        out[b, :, zh * ZS : (zh + 1) * ZS] = results[c]["y"].astype(np.float32)
    return out


def kernel(x, weight):
    from concourse.bass_utils import run_bass_kernel_spmd

    in_maps = _make_in_maps(x, weight)
    nc = _build_program()
    res = run_bass_kernel_spmd(nc, in_maps, list(range(N_CORES)))
    return _gather(res.results)


def _emulate_core(m):
    """Numpy model of one core incl. bf16 rounding of the AT chain."""
    X = np.asarray(m["xd"], np.float32)
    WK = np.asarray(m["wk"], np.float32)
    bf = lambda a: a.astype(ml_dtypes.bfloat16).astype(np.float32)
    y = np.zeros((128, ZS, DHW, DHW), np.float32)
    for zi in ZTILES:
        ps = np.zeros((NK, 128, NT), np.float32)
        for kk in range(NK):
            for p, (dz, dx, lo, hi, kind) in enumerate(KPASSES):
                j = kk * NP_K + p
                r = X[lo:hi, kk, zi + dz : zi + dz + ZT, 0:NW, dx : dx + 24]
                ps[kk] += WK[lo:hi, j].T @ r.reshape(hi - lo, -1)
        m0 = bf(ps[0])
        m5 = bf(ps[5])
        m1, m2, m3, m4 = bf(ps[1]), bf(ps[2]), bf(ps[3]), bf(ps[4])
        a = bf(m1 + m2)
        b_ = bf(m1 - m2)
        pp = bf(m3 + m4)
        q = bf(m3 - m4)
        u = bf(a + pp)
        rows = [bf(u + m0), bf(2 * q + b_), bf(4 * pp + a), bf(bf(8 * q + b_) + m5)]
        yi = np.stack([r.reshape(128, ZT, NW, 24) for r in rows], axis=3)
        y[:, zi : zi + ZT] = yi.reshape(128, ZT, 24, 24)
    return y


if __name__ == "__main__":
    import jax

    sys.path.insert(0, "/root/problem")
    import reference

    cpu = jax.devices("cpu")[0]
    with jax.default_device(cpu):
        inputs = {k: np.asarray(v) for k, v in reference.setup_inputs().items()}
        expected = np.asarray(
            reference.reference(**{k: jax.device_put(v, cpu) for k, v in inputs.items()})
        )
    in_maps = _make_in_maps(inputs["x"], inputs["weight"])
    y = _emulate_core(in_maps[0])
    exp = expected[0][:, 0:ZS]
    err = np.linalg.norm(y - exp) / np.linalg.norm(exp)
    print("emulated core0 rel err:", err)


# revision 56
# speedup vs baseline: 1.9145x; 1.0554x over previous
"""Winograd F(4,3)-y conv3d. Measured ~36.6-37.1us/conv per core
(baseline 56.7us on the same min-slope harness; 68.9us as originally
quoted).

The 39.6->37us step came from two changes: (1) the evacuation-tile
pool had bufs=1 (an SBUF-pressure cut made when an extra input tensor
was resident), creating a cross-engine WAR that delayed PSUM bank
releases on the PE critical path -- dropping the xe tensor (measured
performance-neutral: its 3.35MB DMA trades 1:1 against 24 extra
matmuls) freed 51KB so evacuation tiles double-buffer; (2) ACT copies
ALL six m-terms out of PSUM (bank release rides the fast ACT FIFO)
and DVE pairs them at 2x bf16 rate.

Per-core work: 4 z-tiles x 3 planes, N=432 per matmul (one PSUM bank),
6 winograd m-terms (kk) x 6 passes covering the 9 (dz,dx) taps as
3 x-duals + 3 singles, all from one xd=[T; T(+1x)] tensor -- 144
matmuls/conv. HW ablations: DMA alone ~15us/conv (5.2MB at the HBM
limit), compute pipeline ~30us (144 MMs at ~195-210ns effective incl.
issue/sem overhead); input DMA almost fully hides behind compute.

What mattered on real HW (sim cost model was misleading):
- For_i carries an all-engine barrier + sem reset per iteration, so
  bodies are unrolled 8x inside the loop (`inner`) and weights load
  once per block; per-conv time is the marginal slope.
- HWDGE DMAs execute FIFO per issuing ring: y-store rides the idle
  SWDGE/Pool ring and xe the ACT ring so nothing head-of-line blocks
  the next body's xd chunks on the SP ring. On-chip SBUF->SBUF shift
  copies (tried first) were far slower than shipping both host-shifted
  tensors.
- Pass-major matmul order across the 4 z-tiles: consecutive MMs share
  stationary weights, eliding 3/4 LDWEIGHTS (same-row-group LDW can't
  hide behind a running matmul).
- Inverse transform fused into evacuation: ACT copies m0/m1/m3/m5 out
  of PSUM, DVE forms a/b/p/q with one PSUM operand each (HW limit:
  max one PSUM input per DVE op), scalar_tensor_tensor fuses the
  *2/*4/*8 scaled adds, intermediates bf16 (2x DVE rate). All of this
  overlaps the PE stream completely (measured: removing it changes
  nothing).
"""

import sys

if "/opt/trn_rl_repo" not in sys.path:
    sys.path.insert(0, "/opt/trn_rl_repo")

import ml_dtypes
import numpy as np

CIN, COUT, K = 64, 128, 3
DHW = 24
ZS = 12  # z planes per core
NPL = 14  # input planes incl halo
PW = 26
NW = 6  # y window count (stride 4, size 6)
NK = 6  # winograd m-terms per window
N_CORES = 8
ZT = 3  # z planes per tile
ZTILES = (0, 3, 6, 9)
NT = ZT * NW * 24  # 432 cols per matmul

BT = np.array(
    [
        [4, 0, -5, 0, 1, 0],
        [0, -4, -4, 1, 1, 0],
        [0, 4, -4, -1, 1, 0],
        [0, -2, -1, 2, 1, 0],
        [0, 2, -1, -2, 1, 0],
        [0, 4, 0, -5, 0, 1],
    ],
    np.float32,
)
G = np.array(
    [
        [1 / 4, 0, 0],
        [-1 / 6, -1 / 6, -1 / 6],
        [-1 / 6, 1 / 6, -1 / 6],
        [1 / 24, 1 / 12, 1 / 6],
        [1 / 24, -1 / 12, 1 / 6],
        [0, 0, 1],
    ],
    np.float32,
)

# per-(kk,tile) passes: (dz, dx_ap, lo, hi). All passes read xd=[T; T(+1x)]:
# a [0:128] pass computes taps (dz, dx_ap) + (dz, dx_ap+1); a [0:64] pass
# the single lower tap (dz, dx_ap); a [64:128] pass the single upper tap
# (dz, dx_ap+1) via tile_position (64,0).
KPASSES = (
    (0, 0, 0, 128, "dual"),  # (0,0)+(0,1)
    (1, 0, 0, 128, "dual"),  # (1,0)+(1,1)
    (2, 0, 0, 128, "dual"),  # (2,0)+(2,1)
    (0, 2, 0, 128, "lo"),  # single (0,2): zero upper weights (keeps FWL on)
    (1, 1, 0, 128, "up"),  # single (1,2): upper half only, zero lower weights
    (2, 2, 0, 128, "lo"),  # single (2,2): zero upper weights
)
NP_K = len(KPASSES)  # 6


def _elide_redundant_ldweights(nc):
    n_drop = 0
    for f in nc.m.functions:
        for b in f.blocks:
            last_key = None
            drop = []
            for inst in b.instructions:
                if type(inst).__name__ == "InstLdweights":
                    key = (str(inst.ins[0]), str(inst.perf_mode), str(inst.is_transpose))
                    si = inst.sync_info
                    clean = si is None or (len(si.on_wait) == 0 and len(si.on_update) == 0)
                    if key == last_key and clean:
                        drop.append(inst)
                    else:
                        last_key = key
            for inst in drop:
                b.instructions.remove(inst)
            n_drop += len(drop)
    return n_drop


def _build_program(loop_n=None, unroll=False, inner=1, bodies=1):
    import concourse.tile as tile
    from concourse import bacc, mybir

    BF16 = mybir.dt.bfloat16
    F32 = mybir.dt.float32
    MULT = mybir.AluOpType.mult
    ADD = mybir.AluOpType.add

    nc = bacc.Bacc("TRN2")
    xd_in = nc.declare_dram_parameter("xd", [128, NK, NPL, NW, PW], BF16, isOutput=False)
    wk_in = nc.declare_dram_parameter("wk", [128, NK * NP_K, 128], BF16, isOutput=False)
    y_out = nc.declare_dram_parameter("y", [128, ZS, DHW, DHW], BF16, isOutput=True)

    with tile.TileContext(nc) as tc:
        with (
            tc.tile_pool(name="xw", bufs=1) as xw_pool,
            tc.tile_pool(name="ps", bufs=8, space="PSUM") as ps_pool,
            tc.tile_pool(name="ev", bufs=2) as ev_pool,
            tc.tile_pool(name="ob", bufs=2) as ob_pool,
        ):

            def body(W):
                # 3 chunks of 2 kk each: few dma_starts (fixed cost ~2us each
                # on the ring) but the first matmuls still start early
                xdk = []
                for ck in range(3):
                    xd = xw_pool.tile(
                        [128, 2, NPL, NW, PW], BF16, name=f"xd{ck}", tag=f"xd{ck}"
                    )
                    nc.sync.dma_start(out=xd[:], in_=xd_in[:, 2 * ck : 2 * ck + 2])
                    xdk.append(xd)

                # psum accumulators, one bank per (tile, kk) group, evacuated
                # progressively so 8 banks suffice in kk-major order
                psq = {}
                evt = {}
                for kk in range(NK):
                    for t in range(4):
                        psq[(kk, t)] = ps_pool.tile(
                            [128, 512], F32, name="ps", tag="ps"
                        )
                    # pass-major over the 4 z-tiles: consecutive matmuls share
                    # the same stationary weights, so 3 of every 4 LDWEIGHTS
                    # get elided (same-row-group LDW cannot hide behind a
                    # running matmul, so each unelided one costs ~100ns)
                    for p, (dz, dx, lo, hi, kind) in enumerate(KPASSES):
                        j = kk * NP_K + p
                        for t, zi in enumerate(ZTILES):
                            rhs = xdk[kk // 2][
                                lo:hi, kk % 2, zi + dz : zi + dz + ZT, 0:NW,
                                dx : dx + 24,
                            ]
                            nc.tensor.matmul(
                                psq[(kk, t)][:, :NT],
                                lhsT=W[lo:hi, j, :],
                                rhs=rhs,
                                start=(p == 0),
                                stop=(p == NP_K - 1),
                                skip_group_check=True,
                            )
                    # progressive evacuation: frees the two source banks per op
                    for t in range(4):
                        ps = psq[(kk, t)]

                        def ev(nm, _t=t, dt=BF16):
                            tl = ev_pool.tile(
                                [128, NT], dt, name=f"{nm}{_t}", tag=f"{nm}{_t}"
                            )
                            evt[(nm, _t)] = tl
                            return tl

                        # ACT copies every m-term out of PSUM (bank release
                        # rides the fast ACT FIFO, decoupled from DVE's
                        # queue); DVE then pairs them at 2x bf16 rate
                        mk = ev(f"m{kk}")
                        nc.scalar.copy(mk[:], ps[:, :NT])
                        if kk == 2:
                            a = ev("a")
                            nc.vector.tensor_add(
                                a[:], evt[("m1", t)][:], evt[("m2", t)][:]
                            )
                            b = ev("b")
                            nc.vector.tensor_sub(
                                b[:], evt[("m1", t)][:], evt[("m2", t)][:]
                            )
                        elif kk == 4:
                            pp = ev("p")
                            nc.vector.tensor_add(
                                pp[:], evt[("m3", t)][:], evt[("m4", t)][:]
                            )
                            q = ev("q")
                            nc.vector.tensor_sub(
                                q[:], evt[("m3", t)][:], evt[("m4", t)][:]
                            )

                # final combine per tile into one merged output buffer,
                # then a single y store (one dma_start instead of four)
                ob = ob_pool.tile([128, ZS, NW, 4, 24], BF16, name="ob", tag="ob")
                for t, zi in enumerate(ZTILES):
                    m0, m5, a, b, pp, q = (
                        evt[(nm, t)] for nm in ("m0", "m5", "a", "b", "p", "q")
                    )
                    u = ev_pool.tile([128, NT], BF16, name=f"u{t}", tag=f"u{t}")
                    nc.vector.tensor_add(u[:], a[:], pp[:])
                    nc.vector.tensor_add(ob[:, zi : zi + ZT, :, 0, :], u[:], m0[:])
                    nc.vector.scalar_tensor_tensor(
                        ob[:, zi : zi + ZT, :, 1, :], q[:], 2.0, b[:], MULT, ADD
                    )
                    nc.vector.scalar_tensor_tensor(
                        ob[:, zi : zi + ZT, :, 2, :], pp[:], 4.0, a[:], MULT, ADD
                    )
                    t2 = ev_pool.tile([128, NT], BF16, name=f"t{t}", tag=f"t{t}")
                    nc.vector.scalar_tensor_tensor(t2[:], q[:], 8.0, b[:], MULT, ADD)
                    nc.vector.tensor_add(ob[:, zi : zi + ZT, :, 3, :], t2[:], m5[:])
                # y store on the idle SWDGE/Pool ring so it never head-of-line
                # blocks the next body's xd loads (SP) or W/copies (ACT)
                nc.gpsimd.dma_start(out=y_out[:], in_=ob[:])

            def block(n_bodies):
                # weights are loop-invariant: one load per block
                W = xw_pool.tile([128, NK * NP_K, 128], BF16, name="W", tag="W")
                nc.scalar.dma_start(out=W[:], in_=wk_in[:])
                for _ in range(n_bodies):
                    body(W)

            if loop_n is not None:
                if unroll:
                    for _k in range(loop_n):
                        block(1)
                else:
                    with tc.For_i(0, loop_n, 1) as _i:
                        block(inner)
            else:
                block(bodies)

    nc.finalize()
    _elide_redundant_ldweights(nc)
    return nc


def _wtap(gw, kk, dz, dx):
    return gw[kk, :, :, dz, dx].T


def _transform_w(weight):
    w = np.asarray(weight, np.float32).reshape(COUT, CIN, K, K, K)
    gw = np.einsum("ky,oczyx->koczx", G, w)  # (6, O, C, 3z, 3x)
    wk = np.zeros((128, NK * NP_K, 128), np.float32)
    for kk in range(NK):
        for p, (dz, dx, lo, hi, kind) in enumerate(KPASSES):
            j = kk * NP_K + p
            if kind == "dual":  # (dz, dx) + (dz, dx+1)
                wk[0:64, j] = _wtap(gw, kk, dz, dx)
                wk[64:128, j] = _wtap(gw, kk, dz, dx + 1)
            elif kind == "lo":  # lower single (dz, dx)
                wk[0:64, j] = _wtap(gw, kk, dz, dx)
            else:  # upper single (dz, dx+1); lower rows stay zero
                wk[64:128, j] = _wtap(gw, kk, dz, dx + 1)
    return wk.astype(ml_dtypes.bfloat16)


def _make_in_maps(x, weight):
    wk = _transform_w(weight)
    x = np.asarray(x, np.float32)
    in_maps = []
    for c in range(N_CORES):
        b, zh = divmod(c, 2)
        z0 = zh * ZS
        xpad = np.zeros((CIN, PW, PW, PW), np.float32)
        xpad[:, 1:25, 1:25, 1:25] = x[b]
        win = xpad[:, z0 : z0 + NPL]  # (64, 14, 26, 26)
        # T[c, k, z, w, x] = sum_j BT[k, j] win[c, z, 4w+j, x]
        wmat = np.lib.stride_tricks.sliding_window_view(win, 6, axis=2)[:, :, ::4][
            :, :, :NW
        ]
        T = np.einsum("kj,czwxj->ckzwx", BT, wmat)  # (64, 6, 14, 6, 26)
        X = np.zeros((128, NK, NPL, NW, PW), np.float32)
        X[0:64] = T
        X[64:128, :, :, :, :-1] = T[:, :, :, :, 1:]  # +1x shift
        in_maps.append({"wk": wk, "xd": X.astype(ml_dtypes.bfloat16)})
    return in_maps


def _gather(results):
    out = np.empty((4, COUT, DHW, DHW, DHW), np.float32)
    for c in range(N_CORES):
        b, zh = divmod(c, 2)
        out[b, :, zh * ZS : (zh + 1) * ZS] = results[c]["y"].astype(np.float32)
    return out


def kernel(x, weight):
    from concourse.bass_utils import run_bass_kernel_spmd

    in_maps = _make_in_maps(x, weight)
    nc = _build_program()
    res = run_bass_kernel_spmd(nc, in_maps, list(range(N_CORES)))
    return _gather(res.results)


def _emulate_core(m):
    """Numpy model of one core incl. bf16 rounding of the AT chain."""
    X = np.asarray(m["xd"], np.float32)
    WK = np.asarray(m["wk"], np.float32)
    bf = lambda a: a.astype(ml_dtypes.bfloat16).astype(np.float32)
    y = np.zeros((128, ZS, DHW, DHW), np.float32)
    for zi in ZTILES:
        ps = np.zeros((NK, 128, NT), np.float32)
        for kk in range(NK):
            for p, (dz, dx, lo, hi, kind) in enumerate(KPASSES):
                j = kk * NP_K + p
                r = X[lo:hi, kk, zi + dz : zi + dz + ZT, 0:NW, dx : dx + 24]
                ps[kk] += WK[lo:hi, j].T @ r.reshape(hi - lo, -1)
        m0 = bf(ps[0])
        m5 = bf(ps[5])
        m1, m2, m3, m4 = bf(ps[1]), bf(ps[2]), bf(ps[3]), bf(ps[4])
        a = bf(m1 + m2)
        b_ = bf(m1 - m2)
        pp = bf(m3 + m4)
        q = bf(m3 - m4)
        u = bf(a + pp)
        rows = [bf(u + m0), bf(2 * q + b_), bf(4 * pp + a), bf(bf(8 * q + b_) + m5)]
        yi = np.stack([r.reshape(128, ZT, NW, 24) for r in rows], axis=3)
        y[:, zi : zi + ZT] = yi.reshape(128, ZT, 24, 24)
    return y


if __name__ == "__main__":
    import jax

    sys.path.insert(0, "/root/problem")
    import reference

    cpu = jax.devices("cpu")[0]
    with jax.default_device(cpu):
        inputs = {k: np.asarray(v) for k, v in reference.setup_inputs().items()}
        expected = np.asarray(
            reference.reference(**{k: jax.device_put(v, cpu) for k, v in inputs.items()})
        )
    in_maps = _make_in_maps(inputs["x"], inputs["weight"])
    y = _emulate_core(in_maps[0])
    exp = expected[0][:, 0:ZS]
    err = np.linalg.norm(y - exp) / np.linalg.norm(exp)
    print("emulated core0 rel err:", err)
